# revision 1
# baseline (speedup 1.0000x reference)
"""AttnDecoder kernel for 8 trn2 NeuronCores — transposed-gates design.

Math notes (exact in real arithmetic):
 - The reference's additive attention has no nonlinearity between W1/W2/w3, so
   softmax over s cancels every t-dependent term: attn (and ctx) are
   t-independent. ctx[b] is computed on the host.
 - logits = dec @ Wout[:, :H].T + (ctx @ Wout[:, H:].T + bout); the second
   term is t-independent and is added on the host.
 - Device work: the 2-layer LSTM recurrence (replicated on all 8 cores) and
   the dec-half of the vocab projection (vocab-sharded, 4096 padded cols per
   core).

Layout: everything is "output-transposed" — matmul outputs keep hidden/vocab
dims on partitions and the batch (8) on the free dim, so each recurrent
matmul streams only 8 columns. Gate pre-activations for step t live in one
PSUM tile [128, 128] with col = gate*32 + j*8 + b (j = h-dim block), so the
whole nonlinearity is a handful of [128,32] elementwise ops and h is produced
directly in the layout the next step's matmul consumes.
"""

import numpy as np
import ml_dtypes

B, T, S = 8, 64, 128
V, E, H = 32000, 512, 512
NCORES = 8
VS = V // NCORES   # 4000 real vocab cols per core
VSP = 4096         # padded to 32 chunks of 128
NVC = VSP // 128   # 32 vocab chunks
NTC = 4            # token chunks of 16 steps

_BF16 = ml_dtypes.bfloat16


def _reorder_w(Wih, Whh):
    """[128, 8*2048]: rounds 0-3 = Wih K-chunks, 4-7 = Whh K-chunks.
    col j*512 + g*128 + x  <-  W[g*512 + 128j + x, 128*ki + p]; g-gate rows x2
    (tanh(z) = 2*sigmoid(2z) - 1 lets one Sigmoid call cover all gates)."""
    out = np.zeros((128, 8 * 2048), np.float32)
    for r in range(8):
        Wsrc = Wih if r < 4 else Whh
        ki = r % 4
        blk = Wsrc[:, 128 * ki:128 * (ki + 1)]          # [2048, 128] (gates, p)
        t_ = blk.reshape(4, 4, 128, 128)                # [g, j, x, p]
        t_ = t_.transpose(3, 1, 0, 2)                   # [p, j, g, x]
        out[:, r * 2048:(r + 1) * 2048] = t_.reshape(128, 2048)
    w5 = out.reshape(128, 8, 4, 4, 128)                 # [p, r, j, g, x]
    w5[:, :, :, 2, :] *= 2.0
    return out


def _gate_split(W):
    """[128, 8*2048] round-major -> [ifg 8*1536 | o 8*512] for late o-gate DMA."""
    w5 = W.reshape(128, 8, 4, 4, 128)                  # [p, r, j, g, x]
    ifg = w5[:, :, :, :3].reshape(128, 8 * 1536)       # [p, (r, j, g<3, x)]
    o = w5[:, :, :, 3].reshape(128, 8 * 512)           # [p, (r, j, x)]
    return np.concatenate([ifg, o], axis=1)


def _build_nc():
    import os
    import concourse.bass as bass
    import concourse.bacc as bacc
    import concourse.mybir as mybir
    import concourse.tile as tile

    NO_PROJ = False
    NO_LSTM = False
    PROJ_RATE = 1
    ORDER = "chain"
    HMUL = "dve"
    CCHUNK = 128
    CENG = "alt"
    GSPLIT = True
    BWAIT_MS = 0.0
    CDT = "bf16"

    f32 = mybir.dt.float32
    bf16 = mybir.dt.bfloat16
    AF = mybir.ActivationFunctionType
    OP = mybir.AluOpType

    nc = bacc.Bacc(None, target_bir_lowering=False)
    d = {}
    d["W0"] = nc.dram_tensor("W0", [128, 4 * 2048], bf16, kind="ExternalInput")
    d["W1"] = nc.dram_tensor("W1", [128, 8 * 2048], bf16, kind="ExternalInput")
    d["Wd"] = nc.dram_tensor("Wd", [128, 4 * VSP], bf16, kind="ExternalInput")
    d["ig0"] = nc.dram_tensor("ig0", [128, T * 128], bf16, kind="ExternalInput")
    d["misc"] = nc.dram_tensor("misc", [128, 448], bf16, kind="ExternalInput")
    d["c0T"] = nc.dram_tensor("c0T", [128, 64], bf16,
                              kind="ExternalInput")
    out_d = nc.dram_tensor("out", [128, NTC * VSP], bf16, kind="ExternalOutput")

    with tile.TileContext(nc) as tc:
        with (
            tc.tile_pool(name="const", bufs=1) as cp,
            tc.tile_pool(name="work", bufs=4) as wp,
            tc.tile_pool(name="psA", bufs=3, space="PSUM") as ppA,
            tc.tile_pool(name="psB", bufs=2, space="PSUM") as ppB,
            tc.tile_pool(name="psP", bufs=3, space="PSUM") as ppP,
        ):
            W0s = cp.tile([128, 4 * 2048], bf16, tag="W0s")
            W1s = cp.tile([128, 8 * 2048], bf16, tag="W1s")
            Wds = cp.tile([128, 4 * VSP], bf16, tag="Wds")
            ig0s = cp.tile([128, T * 128], bf16, tag="ig0s")
            misc_sb = cp.tile([128, 448], bf16, tag="misc")
            ids = misc_sb[:, 0:128]
            b1s = misc_sb[:, 128:256]
            dec0T = cp.tile([128, (T + 1) * 32], bf16, tag="dec0T")
            decT = cp.tile([128, (T + 1) * 32], bf16, tag="decT")
            cdt = f32 if CDT == "f32" else bf16
            c_sb = cp.tile([128, 64], cdt, tag="c_sb")
            stage = cp.tile([128, NTC * VSP], bf16, tag="stage")

            # Step-0 inputs first (inject(0,0) + first rec chunk), then weights.
            nc.sync.dma_start(misc_sb[:], d["misc"][:])
            nc.sync.dma_start(W0s[:], d["W0"][:])
            nc.sync.dma_start(W1s[:, 0:8192], d["W1"][:, 0:8192])
            nc.sync.dma_start(c_sb[:], d["c0T"][:])
            nc.sync.dma_start(ig0s[:, 128:512], d["ig0"][:, 128:512])
            nc.sync.dma_start(W1s[:, 8192:16384], d["W1"][:, 8192:16384])
            nc.sync.dma_start(ig0s[:, 512:2048], d["ig0"][:, 512:2048])
            for r in range(1, 4):
                nc.sync.dma_start(ig0s[:, r * 2048:(r + 1) * 2048],
                                  d["ig0"][:, r * 2048:(r + 1) * 2048])
            for r in range(4):
                nc.sync.dma_start(Wds[:, r * VSP:(r + 1) * VSP],
                                  d["Wd"][:, r * VSP:(r + 1) * VSP])

            own = [dec0T, decT]
            pools = [ppA, ppB]
            ps_t = [{}, {}]   # layer -> t -> psum tile
            sg_t = [{}, {}]   # layer -> t -> sigmoid output tile
            cn_t = [{}, {}]   # layer -> t -> new-c tile

            def inject(layer, t):
                ps = pools[layer].tile([128, 128], f32, tag=f"ps{layer}",
                                       name=f"ps{layer}_{t}")
                ps_t[layer][t] = ps
                if layer == 0:
                    src = (misc_sb[:, 320:448] if t == 0
                           else ig0s[:, 128 * t:128 * (t + 1)])
                else:
                    src = b1s[:]
                nc.tensor.matmul(ps[:], src, ids[:], start=True, stop=False)

            def mm_gates(layer, t, gates, stop):
                ps = ps_t[layer][t]
                kinds = ("rec",) if layer == 0 else ("rec", "x")
                for kind in kinds:
                    if kind == "rec":
                        Wr = W0s if layer == 0 else W1s
                        roff = 0 if layer == 0 else 4 * 2048
                        if t == 0:
                            src = misc_sb
                            base = 256 + 32 * layer
                        else:
                            src = own[layer]
                            base = 32 * t
                    else:
                        Wr, roff, src, base = W1s, 0, dec0T, 32 * (t + 1)
                    for k in range(4):
                        hs = src[:, base + 8 * k: base + 8 * k + 8]
                        for j in range(4):
                            for g in gates:
                                last = stop and kind == kinds[-1] and k == 3
                                if not GSPLIT:
                                    last = (last and j == 3 and g == gates[-1])
                                nc.tensor.matmul(
                                    ps[:, g * 32 + j * 8: g * 32 + j * 8 + 8],
                                    Wr[:, roff + k * 2048 + (4 * j + g) * 128:
                                       roff + k * 2048 + (4 * j + g + 1) * 128],
                                    hs, start=False, stop=last,
                                    skip_group_check=GSPLIT)

            def mm_block(layer, t):
                sg = wp.tile([128, 128], cdt, tag=f"sg{layer}")
                sg_t[layer][t] = sg
                ps = ps_t[layer][t]
                if GSPLIT:
                    mm_gates(layer, t, (2, 0, 1), True)
                    nc.scalar.activation(sg[:, 0:96], ps[:, 0:96], AF.Sigmoid)
                    mm_gates(layer, t, (3,), True)
                    nc.scalar.activation(sg[:, 96:128], ps[:, 96:128],
                                         AF.Sigmoid)
                else:
                    mm_gates(layer, t, (0, 1, 2, 3), True)
                    nc.scalar.activation(sg[:], ps[:], AF.Sigmoid)

            def chain_dve3(layer, t):
                # c = sig(f)*c + sig(i)*tanh(zg); tanh(zg) = 2*sig(2zg)-1 and
                # the 2x is baked into the g-gate weights, so with
                # m2h = (sg'-0.5)*si:  c_new = 2*m2h + sig(f)*c.
                sg = sg_t[layer][t]
                cs = c_sb[:, layer * 32:(layer + 1) * 32]
                m2 = wp.tile([128, 32], cdt, tag=f"m2{layer}")
                nc.vector.scalar_tensor_tensor(m2[:], sg[:, 64:96], 0.5,
                                               sg[:, 0:32],
                                               OP.subtract, OP.mult)
                m1 = wp.tile([128, 32], cdt, tag=f"m1{layer}")
                nc.vector.tensor_mul(m1[:], sg[:, 32:64], cs)
                nc.vector.tensor_add(cs, m2[:], m1[:])

            def chain_tanh(layer, t):
                cs = c_sb[:, layer * 32:(layer + 1) * 32]
                cn = wp.tile([128, 32], cdt, tag=f"cn{layer}")
                cn_t[layer][t] = cn
                nc.scalar.activation(cn[:], cs, AF.Tanh, scale=2.0)

            def chain_hmul(layer, t):
                sg = sg_t[layer][t]
                if HMUL == "pool" or (HMUL == "split" and layer == 1):
                    eng = nc.gpsimd
                else:
                    eng = nc.vector
                eng.tensor_mul(own[layer][:, 32 * (t + 1):32 * (t + 2)],
                               sg[:, 96:128], cn_t[layer][t][:])

            decv = decT.rearrange("p (s c) -> p s c", c=32)
            proj_q = []           # pending (vc, t0, nt, col) projection tasks
            copies_done = [0] * NTC
            tc3_done = [0, 0, 0, 0]   # tc3 copies done per 2-vcg range

            def emit_proj(n):
                if not hasattr(emit_proj, "rr"):
                    emit_proj.rr = 0
                for _ in range(min(n, len(proj_q))):
                    vcg, t0, nt, half = proj_q.pop(0)
                    tcb = t0 // 16
                    w = nt * 8
                    psP = ppP.tile([128, 512], f32, tag="psP",
                                   name=f"psP_{t0}_{vcg}_{half}")
                    for vi in range(4):
                        vc = 4 * vcg + vi
                        for k in range(4):
                            nc.tensor.matmul(
                                psP[:, vi * 128: vi * 128 + w],
                                Wds[:, k * VSP + vc * 128:
                                    k * VSP + (vc + 1) * 128],
                                decv[:, t0 + 1: t0 + 1 + nt,
                                     8 * k:8 * k + 8],
                                start=(k == 0), stop=(k == 3))
                    base = tcb * VSP + vcg * 512

                    def do_copy(dst, src):
                        emit_proj.rr += 1
                        if CENG == "act":
                            use_act = True
                        elif CENG == "alt":
                            use_act = bool(emit_proj.rr % 2)
                        elif CENG == "dda":
                            use_act = (emit_proj.rr % 3 == 0)
                        elif CENG == "daa":
                            use_act = (emit_proj.rr % 3 != 0)
                        else:
                            use_act = False
                        if use_act:
                            nc.scalar.activation(dst, src, AF.Copy)
                        else:
                            nc.vector.tensor_copy(dst, src)

                    if w == 128:
                        for c0 in range(0, 512, CCHUNK):
                            do_copy(stage[:, base + c0:base + c0 + CCHUNK],
                                    psP[:, c0:c0 + CCHUNK])
                    else:
                        pv = psP.rearrange("p (v c) -> p v c", c=128)
                        sv = stage.rearrange("p (v c) -> p v c", c=128)
                        do_copy(sv[:, (base + half * 64) // 128:
                                    (base + half * 64) // 128 + 4,
                                    half * 64:half * 64 + 64],
                                pv[:, 0:4, 0:64])
                    if tcb < 3:
                        copies_done[tcb] += nt * 4
                        if copies_done[tcb] == 16 * NVC:
                            nc.sync.dma_start(
                                out_d[:, tcb * VSP:(tcb + 1) * VSP],
                                stage[:, tcb * VSP:(tcb + 1) * VSP])
                    else:
                        rg = vcg // 2
                        tc3_done[rg] += nt * 4
                        if tc3_done[rg] == 4 * 32:
                            o0 = 3 * VSP + rg * 1024
                            nc.sync.dma_start(out_d[:, o0:o0 + 1024],
                                              stage[:, o0:o0 + 1024])

            if not NO_LSTM:
                inject(0, 0)
                for t in range(T):
                    s = t - 1   # layer-1 step handled this superstep
                    if ORDER == "chain":
                        mm_block(0, t)
                        chain_dve3(0, t)
                        chain_tanh(0, t)
                        chain_hmul(0, t)
                        if s >= 0:
                            import contextlib
                            gate = (tc.tile_wait_until(BWAIT_MS)
                                    if s == 0 and BWAIT_MS > 0
                                    else contextlib.nullcontext())
                            with gate:
                                inject(1, s)
                                mm_block(1, s)
                            chain_dve3(1, s)
                            chain_tanh(1, s)
                            chain_hmul(1, s)
                        if t + 1 < T:
                            inject(0, t + 1)
                    else:
                        mm_block(0, t)
                        if s >= 0:
                            inject(1, s)
                            mm_block(1, s)
                        if t + 1 < T:
                            inject(0, t + 1)
                        chain_dve3(0, t)
                        if s >= 0:
                            chain_dve3(1, s)
                        chain_tanh(0, t)
                        if s >= 0:
                            chain_tanh(1, s)
                        chain_hmul(0, t)
                        if s >= 0:
                            chain_hmul(1, s)
                    if not NO_PROJ:
                        if t >= 17 and (t - 17) % 16 == 0 and t < 60:
                            tcb = (t - 17) // 16
                            proj_q.extend((vcg, 16 * tcb, 16, 0)
                                          for vcg in range(8))
                        if t == 58:
                            proj_q.extend((vcg, 48, 8, 0)
                                          for vcg in range(8))
                        emit_proj(PROJ_RATE if t < 58 else 2)
                # drain layer-1 step T-1
                s = T - 1
                inject(1, s)
                mm_block(1, s)
                chain_dve3(1, s)
                chain_tanh(1, s)
                chain_hmul(1, s)
            if not NO_PROJ:
                proj_q.extend((vcg, 56, 8, 1) for vcg in range(8))
                emit_proj(len(proj_q))
            if NO_PROJ:
                # minimal output so the module still has a consumer
                psP = ppP.tile([128, 128], f32, tag="psP")
                nc.tensor.matmul(psP[:], ids[:], ids[:], start=True, stop=True)
                nc.gpsimd.tensor_copy(stage[:, 0:128], psP[:])
                nc.sync.dma_start(out_d[:, 0:128], stage[:, 0:128])
            if NO_LSTM and not NO_PROJ:
                pass
    nc.finalize()
    return nc


_NC_CACHE = None


def _get_nc():
    global _NC_CACHE
    if _NC_CACHE is None:
        _NC_CACHE = _build_nc()
    return _NC_CACHE


import os


def _host_inputs(input_ids, enc_output, h0, c0, emb, Wih0, Whh0, bih0, bhh0,
                 Wih1, Whh1, bih1, bhh1, W1, b1, W2, b2, w3, b3, Wout, bout):
    f32 = np.float32
    x = np.asarray(emb, f32)[np.asarray(input_ids).astype(np.int64)]  # [B,T,E]

    # Layer-0 input projection on the host (exact), g-gate x2, bias folded in.
    ig0 = x @ np.asarray(Wih0, f32).T + (np.asarray(bih0, f32)
                                         + np.asarray(bhh0, f32))   # [B,T,2048]
    ig0 = ig0.reshape(B, T, 4, 4, 128)          # [b,t,g,j,x]
    ig0[:, :, 2] *= 2.0
    ig0T = ig0.transpose(2, 3, 0, 1, 4).reshape(128, T * 128)  # [(g,j,b),(t,x)]

    b1v = (np.asarray(bih1, f32) + np.asarray(bhh1, f32)).reshape(4, 4, 128)
    b1v = b1v.copy()
    b1v[2] *= 2.0                               # [g,j,x]
    b1T = np.broadcast_to(b1v[:, :, None, :], (4, 4, 8, 128)).reshape(128, 128)

    def h0T(hl):
        return hl.T.reshape(4, 128, 8).transpose(1, 0, 2).reshape(128, 32)

    c0a = (np.asarray(c0, f32) * 0.5).reshape(2, 8, 4, 128)
    c0T = c0a.transpose(3, 0, 2, 1).reshape(128, 64)  # [x, (layer,j,b)]

    # collapsed attention (exact in real arithmetic; see module docstring)
    u = np.asarray(W2, f32).T @ np.asarray(w3, f32)[0]
    ue = np.asarray(W1, f32)[:, :H].T @ u
    sc = np.asarray(enc_output, f32) @ ue                  # [B,S]
    sc = sc - sc.max(-1, keepdims=True)
    a = np.exp(sc)
    a /= a.sum(-1, keepdims=True)
    ctxh = np.einsum('bs,bsh->bh', a, np.asarray(enc_output, f32))  # [B,H]

    Wo_full = np.asarray(Wout, f32)                        # [V, 2H]
    bo_full = np.asarray(bout, f32)
    # t-independent half of the projection, added on the host
    ctxadd = ctxh @ Wo_full[:, H:].T + bo_full             # [B, V]

    Wrec = _reorder_w(np.asarray(Wih0, f32), np.asarray(Whh0, f32))
    misc = np.concatenate([np.eye(128, dtype=f32), b1T,
                           h0T(np.asarray(h0, f32)[0]),
                           h0T(np.asarray(h0, f32)[1]),
                           ig0T[:, 0:128]], axis=1)
    base = {
        "W0": np.ascontiguousarray(Wrec[:, 4 * 2048:]).astype(_BF16),
        "W1": _reorder_w(np.asarray(Wih1, f32),
                         np.asarray(Whh1, f32)).astype(_BF16),
        "ig0": ig0T.astype(_BF16),
        "misc": misc.astype(_BF16),
        "c0T": c0T.astype(_BF16),
    }
    Wd_pad = np.zeros((NCORES * VSP, H), f32)
    Wd_pad[:V] = Wo_full[:, :H]
    maps = []
    for k in range(NCORES):
        lo = k * VS
        sh = np.zeros((VSP, H), f32)
        n = min(VSP, V - lo)
        sh[:n] = Wo_full[lo:lo + n, :H]
        t_ = sh.reshape(NVC, 128, 4, 128).transpose(3, 2, 0, 1)  # [p,k,vc,m]
        m = dict(base)
        m["Wd"] = np.ascontiguousarray(t_.reshape(128, 4 * VSP)).astype(_BF16)
        maps.append(m)
    return maps, ctxadd


def kernel(**inputs):
    from concourse.bass_utils import run_bass_kernel_spmd
    nc = _get_nc()
    maps, ctxadd = _host_inputs(**inputs)
    res = run_bass_kernel_spmd(nc, maps, list(range(NCORES))).results
    full = np.zeros((B, T, V), np.float32)
    for k in range(NCORES):
        o = np.asarray(res[k]["out"], np.float32)
        o = o.reshape(128, NTC, NVC, 16, 8)        # [x, tc, vc, tl, b]
        o = o.transpose(4, 1, 3, 2, 0).reshape(B, T, VSP)
        n = min(VS, V - k * VS)
        full[:, :, k * VS:k * VS + n] = o[:, :, :n]
    full += ctxadd[:, None, :]
    return full



# revision 21
# speedup vs baseline: 1.0671x; 1.0671x over previous
"""AttnDecoder kernel for 8 trn2 NeuronCores — latency-optimized chain design.

Math notes (exact in real arithmetic):
 - The reference's additive attention has no nonlinearity between W1/W2/w3, so
   softmax over s cancels every t-dependent term: attn (and ctx) are
   t-independent. ctx[b] is computed on the host.
 - logits = dec @ Wout[:, :H].T + (ctx @ Wout[:, H:].T + bout); the second
   term is t-independent and is added on the host.
 - Device work: the 2-layer LSTM recurrence (replicated on all 8 cores) and
   the dec-half of the vocab projection (vocab-sharded, 4096 padded cols per
   core).

Layout: everything is "output-transposed" — matmul outputs keep hidden/vocab
dims on partitions and the batch (8) on the free dim. Gate pre-activations for
step t live in one PSUM tile [128, 128] with col = gate*32 + j*8 + b
(j = h-dim block); one Sigmoid covers all four gates (tanh(z) = 2*sig(2z)-1
with the 2x baked into the g-gate weights/inputs).

Schedule: the two layers' recurrences are independent chains (layer 1 step
s = t-1 only needs superstep t-1 outputs), interleaved per-engine in data
arrival order so neither blocks the other on the in-order queues:
  PE : rec0(t) | x1(s) | rec1(s) | inject(t+1) | proj matmuls
  Act: sig0(t) | sig1(s) | tanh0(t) | tanh1(s)
  DVE: c-update0(t) | c-update1(s) | hmul0(t) | hmul1(s)
  Pool: projection PSUM->SBUF copies (keeps Act/DVE free for the chain)
Recurrent weights are fp8-e4m3 (halves the weight-load DMA that gates the
layer-1 chain start); activations stay bf16.
"""

import numpy as np
import ml_dtypes

B, T, S = 8, 64, 128
V, E, H = 32000, 512, 512
NCORES = 8
VS = V // NCORES   # 4000 real vocab cols per core
VSP = 4096         # padded to 32 chunks of 128
NVC = VSP // 128   # 32 vocab chunks

_BF16 = ml_dtypes.bfloat16
_F8 = ml_dtypes.float8_e4m3fn
USE_FP8 = False

# Projection task list, in emission order: (vcg0, nvcg, t0, nt).
# Each task fills one PSUM tile with logits for vocab chunks
# [4*vcg0, 4*(vcg0+nvcg)) and decoder steps [t0, t0+nt), laid out
# col = vc_local*(nt*8) + tl*8 + b, then DMAs it straight to DRAM slot
# task_idx*512 (f32). Host unscrambles. Tail tasks pack several vocab
# groups so the end-of-program DMA burst stays short.
TASKS = (
    [(vcg, 1, 0, 16) for vcg in range(8)]
    + [(vcg, 1, 16, 16) for vcg in range(8)]
    + [(vcg, 1, 32, 16) for vcg in range(8)]
    + [(2 * g, 2, 48, 8) for g in range(4)]
    + [(4 * g, 4, 56, 4) for g in range(2)]
    + [(4 * g, 4, 60, 2) for g in range(2)]
    + [(0, 8, 62, 1), (0, 8, 63, 1)]
)
# earliest superstep at which each task's decT inputs exist
TASK_AVAIL = ([17] * 8 + [33] * 8 + [49] * 8 + [57] * 4 + [61] * 2
              + [63] * 2 + [64, 65])


def _reorder_w(Wih, Whh):
    """[128, 8*2048]: rounds 0-3 = Wih K-chunks, 4-7 = Whh K-chunks.
    col j*512 + g*128 + x  <-  W[g*512 + 128j + x, 128*ki + p]; g-gate rows x2
    (tanh(z) = 2*sigmoid(2z) - 1 lets one Sigmoid call cover all gates)."""
    out = np.zeros((128, 8 * 2048), np.float32)
    for r in range(8):
        Wsrc = Wih if r < 4 else Whh
        ki = r % 4
        blk = Wsrc[:, 128 * ki:128 * (ki + 1)]          # [2048, 128] (gates, p)
        t_ = blk.reshape(4, 4, 128, 128)                # [g, j, x, p]
        t_ = t_.transpose(3, 1, 0, 2)                   # [p, j, g, x]
        out[:, r * 2048:(r + 1) * 2048] = t_.reshape(128, 2048)
    w5 = out.reshape(128, 8, 4, 4, 128)                 # [p, r, j, g, x]
    w5[:, :, :, 2, :] *= 2.0
    return out


def _build_nc():
    import concourse.bass as bass
    import concourse.bacc as bacc
    import concourse.mybir as mybir
    import concourse.tile as tile

    f32 = mybir.dt.float32
    bf16 = mybir.dt.bfloat16
    f8 = mybir.dt.float8e4 if USE_FP8 else mybir.dt.bfloat16
    AF = mybir.ActivationFunctionType
    OP = mybir.AluOpType

    nc = bacc.Bacc(None, target_bir_lowering=False)
    d = {}
    d["W0"] = nc.dram_tensor("W0", [128, 4 * 2048], f8, kind="ExternalInput")
    d["W1"] = nc.dram_tensor("W1", [128, 8 * 2048], f8, kind="ExternalInput")
    d["Wd"] = nc.dram_tensor("Wd", [128, 4 * VSP], bf16, kind="ExternalInput")
    d["ig0"] = nc.dram_tensor("ig0", [128, T * 128], bf16, kind="ExternalInput")
    d["misc"] = nc.dram_tensor("misc", [128, 448], bf16, kind="ExternalInput")
    d["c0T"] = nc.dram_tensor("c0T", [128, 64], bf16, kind="ExternalInput")
    out_d = nc.dram_tensor("out", [128, T * 256], bf16,
                           kind="ExternalOutput")

    with tile.TileContext(nc) as tc:
        with (
            tc.tile_pool(name="const", bufs=1) as cp,
            tc.tile_pool(name="work", bufs=4) as wp,
            tc.tile_pool(name="psA", bufs=2, space="PSUM") as ppA,
            tc.tile_pool(name="psB", bufs=2, space="PSUM") as ppB,
            tc.tile_pool(name="psP", bufs=3, space="PSUM") as ppP,
        ):
            W0s = cp.tile([128, 4 * 2048], f8, tag="W0s")
            W1s = cp.tile([128, 8 * 2048], f8, tag="W1s")
            Wds = cp.tile([128, 4 * VSP], bf16, tag="Wds")
            ig0s = cp.tile([128, T * 128], bf16, tag="ig0s")
            misc_sb = cp.tile([128, 448], bf16, tag="misc")
            ids = misc_sb[:, 0:128]
            b1s = misc_sb[:, 128:256]
            dec0T = cp.tile([128, (T + 1) * 32], bf16, tag="dec0T")
            decT = cp.tile([128, (T + 1) * 32], bf16, tag="decT")
            c_sb = cp.tile([128, 64], bf16, tag="c_sb")
            stage = cp.tile([128, T * 256], bf16, tag="stage")

            # Step-0 inputs first, then weights in consumption order.
            nc.sync.dma_start(misc_sb[:], d["misc"][:])
            nc.sync.dma_start(c_sb[:], d["c0T"][:])
            nc.sync.dma_start(ig0s[:, 128:512], d["ig0"][:, 128:512])
            for r in range(4):
                nc.sync.dma_start(W0s[:, r * 2048:(r + 1) * 2048],
                                  d["W0"][:, r * 2048:(r + 1) * 2048])
            for r in range(4):
                nc.sync.dma_start(W1s[:, r * 4096:(r + 1) * 4096],
                                  d["W1"][:, r * 4096:(r + 1) * 4096])
            nc.sync.dma_start(ig0s[:, 512:2048], d["ig0"][:, 512:2048])
            for r in range(1, 4):
                nc.sync.dma_start(ig0s[:, r * 2048:(r + 1) * 2048],
                                  d["ig0"][:, r * 2048:(r + 1) * 2048])
            for r in range(4):
                nc.sync.dma_start(Wds[:, r * VSP:(r + 1) * VSP],
                                  d["Wd"][:, r * VSP:(r + 1) * VSP])

            own = [dec0T, decT]
            pools = [ppA, ppB]
            ps_t = [{}, {}]   # layer -> t -> psum tile
            sg_t = [{}, {}]   # layer -> t -> sigmoid output tile
            cn_t = [{}, {}]   # layer -> t -> new-c tile

            def inject(layer, t):
                ps = pools[layer].tile([128, 128], f32, tag=f"ps{layer}",
                                       name=f"ps{layer}_{t}")
                ps_t[layer][t] = ps
                if layer == 0:
                    src = (misc_sb[:, 320:448] if t == 0
                           else ig0s[:, 128 * t:128 * (t + 1)])
                else:
                    src = b1s[:]
                nc.tensor.matmul(ps[:], src, ids[:], start=True, stop=False,
                                 skip_group_check=True)

            def mm_x(s):
                # layer-1 input-side matmuls (dec0 -> gates); off the
                # critical path (dec0T[s+1] is ready before rec1(s) runs).
                ps = ps_t[1][s]
                for k in range(4):
                    hs = dec0T[:, 32 * (s + 1) + 8 * k: 32 * (s + 1) + 8 * k + 8]
                    for j in range(4):
                        for g in range(4):
                            mw = k * 2048 + (4 * j + g) * 128
                            nc.tensor.matmul(
                                ps[:, g * 32 + j * 8: g * 32 + j * 8 + 8],
                                W1s[:, mw: mw + 128],
                                hs, start=False, stop=False,
                                skip_group_check=True)

            def mm_rec(layer, t):
                ps = ps_t[layer][t]
                Wr = W0s if layer == 0 else W1s
                roff = 0 if layer == 0 else 4 * 2048
                if t == 0:
                    src = misc_sb
                    base = 256 + 32 * layer
                else:
                    src = own[layer]
                    base = 32 * t
                for k in range(4):
                    hs = src[:, base + 8 * k: base + 8 * k + 8]
                    for j in range(4):
                        for g in range(4):
                            last = (k == 3 and j == 3 and g == 3)
                            mw = roff + k * 2048 + (4 * j + g) * 128
                            nc.tensor.matmul(
                                ps[:, g * 32 + j * 8: g * 32 + j * 8 + 8],
                                Wr[:, mw: mw + 128],
                                hs, start=False, stop=last,
                                skip_group_check=True)

            def sig(layer, t):
                sg = wp.tile([128, 128], bf16, tag=f"sg{layer}")
                sg_t[layer][t] = sg
                nc.scalar.activation(sg[:], ps_t[layer][t][:], AF.Sigmoid)

            def cupd(layer, t):
                # c = sig(f)*c + sig(i)*tanh(zg); tanh(zg) = 2*sig(2zg)-1 and
                # the 2x is baked into the g-gate weights, so with
                # m2 = (sg'-0.5)*si:  c_new/2 = m2 + sig(f)*(c/2).
                sg = sg_t[layer][t]
                cs = c_sb[:, layer * 32:(layer + 1) * 32]
                m2 = wp.tile([128, 32], bf16, tag=f"m2{layer}")
                nc.vector.scalar_tensor_tensor(m2[:], sg[:, 64:96], 0.5,
                                               sg[:, 0:32],
                                               OP.subtract, OP.mult)
                m1 = wp.tile([128, 32], bf16, tag=f"m1{layer}")
                nc.vector.tensor_mul(m1[:], sg[:, 32:64], cs)
                nc.vector.tensor_add(cs, m2[:], m1[:])

            def ctanh(layer, t):
                cs = c_sb[:, layer * 32:(layer + 1) * 32]
                cn = wp.tile([128, 32], bf16, tag=f"cn{layer}")
                cn_t[layer][t] = cn
                nc.scalar.activation(cn[:], cs, AF.Tanh, scale=2.0)

            def hmul(layer, t):
                sg = sg_t[layer][t]
                nc.vector.tensor_mul(own[layer][:, 32 * (t + 1):32 * (t + 2)],
                                     sg[:, 96:128], cn_t[layer][t][:])

            # ---------- projection ----------
            # stage col = t*256 + vc*8 + b (t-major: tail regions DMA early)
            decv = decT.rearrange("p (s c) -> p s c", c=32)
            stg = stage.rearrange("p (t v b) -> p v t b", t=T, v=NVC, b=8)
            pq = list(TASKS)
            emit_proj_idx = [0]
            copy_q = []     # pending (psP, vcg0, nvcg, t0, nt, vl0, vl1)
            # DMA regions [t0, t1, pieces_needed, pieces_done]
            regions = [[0, 16, 16, 0], [16, 32, 16, 0], [32, 48, 16, 0],
                       [48, 56, 8, 0], [56, 60, 4, 0], [60, 62, 4, 0],
                       [62, 63, 2, 0], [63, 64, 2, 0]]

            def emit_proj(n):
                for _ in range(n):
                    if emit_proj_idx[0] >= len(pq):
                        return
                    i = emit_proj_idx[0]
                    emit_proj_idx[0] += 1
                    vcg0, nvcg, t0, nt = pq[i]
                    w = nt * 8
                    nvc = 4 * nvcg
                    psP = ppP.tile([128, 512], f32, tag="psP",
                                   name=f"psP_{t0}_{vcg0}")
                    nh = 2 if nt > 8 else 1     # split N to bound PE HOL delay
                    for vl in range(nvc):
                        vc = 4 * vcg0 + vl
                        for k in range(4):
                            for h2 in range(nh):
                                s0 = t0 + 1 + (nt // nh) * h2
                                sn = nt // nh
                                nc.tensor.matmul(
                                    psP[:, vl * w + sn * 8 * h2:
                                        vl * w + sn * 8 * (h2 + 1)],
                                    Wds[:, k * VSP + vc * 128:
                                        k * VSP + (vc + 1) * 128],
                                    decv[:, s0: s0 + sn, 8 * k:8 * k + 8],
                                    start=(k == 0 and h2 == 0),
                                    stop=(k == 3 and h2 == nh - 1),
                                    skip_group_check=True)
                    # two ~256-col copy pieces per task, emitted later at
                    # engine-idle points of the chain schedule
                    copy_q.append((psP, vcg0, nvcg, t0, nt, 0, nvc // 2))
                    copy_q.append((psP, vcg0, nvcg, t0, nt, nvc // 2, nvc))

            def emit_copy(n, eng):
                for _ in range(n):
                    if not copy_q:
                        return
                    psP, vcg0, nvcg, t0, nt, vl0, vl1 = copy_q.pop(0)
                    pv4 = psP.rearrange("p (v t b) -> p v t b", t=nt, b=8)
                    src = pv4[:, vl0:vl1, :, :]
                    dst = stg[:, 4 * vcg0 + vl0:4 * vcg0 + vl1, t0:t0 + nt, :]
                    if eng == "act":
                        nc.scalar.activation(dst, src, AF.Copy)
                    else:
                        nc.vector.tensor_copy(dst, src)
                    for reg in regions:
                        if reg[0] <= t0 < reg[1]:
                            reg[3] += 1
                            if reg[3] == reg[2]:
                                nc.sync.dma_start(
                                    out_d[:, reg[0] * 256:reg[1] * 256],
                                    stage[:, reg[0] * 256:reg[1] * 256])

            # ---------- main loop ----------
            inject(0, 0)
            for t in range(T):
                s = t - 1   # layer-1 step handled this superstep
                mm_rec(0, t)
                if s >= 0:
                    mm_x(s)
                    mm_rec(1, s)
                if t + 1 < T:
                    inject(0, t + 1)
                inject(1, t)
                sig(0, t)
                if s >= 0:
                    sig(1, s)
                cupd(0, t)
                ctanh(0, t)
                if s >= 0:
                    cupd(1, s)
                    ctanh(1, s)
                hmul(0, t)
                if len(copy_q) > 2:
                    emit_copy(1, "act")
                if s >= 0:
                    hmul(1, s)
                emit_copy(2, "dve")
                # projection task feed (keeps PE busy in the chain gaps);
                # gated on TASK_AVAIL so queued matmuls never head-of-line
                # block the PE waiting for future decT blocks.
                budget = 2 if t == 63 else 1
                while (budget and emit_proj_idx[0] < len(pq)
                       and TASK_AVAIL[emit_proj_idx[0]] <= t):
                    emit_proj(1)
                    budget -= 1
            # drain layer-1 step T-1
            s = T - 1
            mm_x(s)
            mm_rec(1, s)
            emit_proj(1)            # (62,1): data completes mid-drain
            sig(1, s)
            emit_copy(1, "act")
            emit_copy(1, "dve")
            cupd(1, s)
            ctanh(1, s)
            hmul(1, s)
            emit_proj(len(pq) - emit_proj_idx[0])
            while copy_q:
                emit_copy(1, "dve")
                emit_copy(1, "act")
    nc.finalize()
    return nc


_NC_CACHE = None


def _get_nc():
    global _NC_CACHE
    if _NC_CACHE is None:
        _NC_CACHE = _build_nc()
    return _NC_CACHE


def _host_inputs(input_ids, enc_output, h0, c0, emb, Wih0, Whh0, bih0, bhh0,
                 Wih1, Whh1, bih1, bhh1, W1, b1, W2, b2, w3, b3, Wout, bout):
    f32 = np.float32
    x = np.asarray(emb, f32)[np.asarray(input_ids).astype(np.int64)]  # [B,T,E]

    # Layer-0 input projection on the host (exact), g-gate x2, bias folded in.
    ig0 = x @ np.asarray(Wih0, f32).T + (np.asarray(bih0, f32)
                                         + np.asarray(bhh0, f32))   # [B,T,2048]
    ig0 = ig0.reshape(B, T, 4, 4, 128)          # [b,t,g,j,x]
    ig0[:, :, 2] *= 2.0
    ig0T = ig0.transpose(2, 3, 0, 1, 4).reshape(128, T * 128)  # [(g,j,b),(t,x)]

    b1v = (np.asarray(bih1, f32) + np.asarray(bhh1, f32)).reshape(4, 4, 128)
    b1v = b1v.copy()
    b1v[2] *= 2.0                               # [g,j,x]
    b1T = np.broadcast_to(b1v[:, :, None, :], (4, 4, 8, 128)).reshape(128, 128)

    def h0T(hl):
        return hl.T.reshape(4, 128, 8).transpose(1, 0, 2).reshape(128, 32)

    c0a = (np.asarray(c0, f32) * 0.5).reshape(2, 8, 4, 128)
    c0T = c0a.transpose(3, 0, 2, 1).reshape(128, 64)  # [x, (layer,j,b)]

    # collapsed attention (exact in real arithmetic; see module docstring)
    u = np.asarray(W2, f32).T @ np.asarray(w3, f32)[0]
    ue = np.asarray(W1, f32)[:, :H].T @ u
    sc = np.asarray(enc_output, f32) @ ue                  # [B,S]
    sc = sc - sc.max(-1, keepdims=True)
    a = np.exp(sc)
    a /= a.sum(-1, keepdims=True)
    ctxh = np.einsum('bs,bsh->bh', a, np.asarray(enc_output, f32))  # [B,H]

    Wo_full = np.asarray(Wout, f32)                        # [V, 2H]
    bo_full = np.asarray(bout, f32)
    # t-independent half of the projection, added on the host
    ctxadd = ctxh @ Wo_full[:, H:].T + bo_full             # [B, V]

    Wrec = _reorder_w(np.asarray(Wih0, f32), np.asarray(Whh0, f32))
    misc = np.concatenate([np.eye(128, dtype=f32), b1T,
                           h0T(np.asarray(h0, f32)[0]),
                           h0T(np.asarray(h0, f32)[1]),
                           ig0T[:, 0:128]], axis=1)
    base = {
        "W0": np.ascontiguousarray(Wrec[:, 4 * 2048:]).astype(_F8 if USE_FP8 else _BF16),
        "W1": _reorder_w(np.asarray(Wih1, f32),
                         np.asarray(Whh1, f32)).astype(_F8 if USE_FP8 else _BF16),
        "ig0": ig0T.astype(_BF16),
        "misc": misc.astype(_BF16),
        "c0T": c0T.astype(_BF16),
    }
    maps = []
    for k in range(NCORES):
        lo = k * VS
        sh = np.zeros((VSP, H), f32)
        n = min(VSP, V - lo)
        sh[:n] = Wo_full[lo:lo + n, :H]
        t_ = sh.reshape(NVC, 128, 4, 128).transpose(3, 2, 0, 1)  # [p,k,vc,m]
        m = dict(base)
        m["Wd"] = np.ascontiguousarray(t_.reshape(128, 4 * VSP)).astype(_BF16)
        maps.append(m)
    return maps, ctxadd


def kernel(**inputs):
    from concourse.bass_utils import run_bass_kernel_spmd
    nc = _get_nc()
    maps, ctxadd = _host_inputs(**inputs)
    res = run_bass_kernel_spmd(nc, maps, list(range(NCORES))).results
    full = np.zeros((B, T, V), np.float32)
    for k in range(NCORES):
        o = np.asarray(res[k]["out"], np.float32)   # [128, T*256]
        o = o.reshape(128, T, NVC, 8)               # [x, t, vc, b]
        o = o.transpose(3, 1, 2, 0).reshape(B, T, VSP)
        n = min(VS, V - k * VS)
        full[:, :, k * VS:k * VS + n] = o[:, :, :n]
    full += ctxadd[:, None, :]
    return full


# revision 23
# speedup vs baseline: 1.1116x; 1.0417x over previous
"""AttnDecoder kernel for 8 trn2 NeuronCores — latency-optimized chain design.

Math notes (exact in real arithmetic):
 - The reference's additive attention has no nonlinearity between W1/W2/w3, so
   softmax over s cancels every t-dependent term: attn (and ctx) are
   t-independent. ctx[b] is computed on the host.
 - logits = dec @ Wout[:, :H].T + (ctx @ Wout[:, H:].T + bout); the second
   term is t-independent and is added on the host.
 - Device work: the 2-layer LSTM recurrence (replicated on all 8 cores) and
   the dec-half of the vocab projection (vocab-sharded, 4096 padded cols per
   core).

Layout: everything is "output-transposed" — matmul outputs keep hidden/vocab
dims on partitions and the batch (8) on the free dim. Gate pre-activations for
step t live in one PSUM tile [128, 128] with col = gate*32 + j*8 + b
(j = h-dim block); one Sigmoid covers all four gates (tanh(z) = 2*sig(2z)-1
with the 2x baked into the g-gate weights/inputs).

Schedule: the two layers' recurrences are independent chains (layer 1 step
s = t-1 only needs superstep t-1 outputs), interleaved per-engine in data
arrival order so neither blocks the other on the in-order queues:
  PE : rec0(t) | x1(s) | rec1(s) | inject(t+1) | proj matmuls
  Act: sig0(t) | sig1(s) | tanh0(t) | tanh1(s)
  DVE: c-update0(t) | c-update1(s) | hmul0(t) | hmul1(s)
  Pool: projection PSUM->SBUF copies (keeps Act/DVE free for the chain)
Recurrent weights are fp8-e4m3 (halves the weight-load DMA that gates the
layer-1 chain start); activations stay bf16.
"""

import numpy as np
import ml_dtypes

B, T, S = 8, 64, 128
V, E, H = 32000, 512, 512
NCORES = 8
VS = V // NCORES   # 4000 real vocab cols per core
VSP = 4096         # padded to 32 chunks of 128
NVC = VSP // 128   # 32 vocab chunks

_BF16 = ml_dtypes.bfloat16
_F8 = ml_dtypes.float8_e4m3fn
USE_FP8 = True

# Projection task list, in emission order: (vcg0, nvcg, t0, nt).
# Each task fills one PSUM tile with logits for vocab chunks
# [4*vcg0, 4*(vcg0+nvcg)) and decoder steps [t0, t0+nt), laid out
# col = vc_local*(nt*8) + tl*8 + b, then DMAs it straight to DRAM slot
# task_idx*512 (f32). Host unscrambles. Tail tasks pack several vocab
# groups so the end-of-program DMA burst stays short.
TASKS = (
    [(vcg, 1, 0, 16) for vcg in range(8)]
    + [(vcg, 1, 16, 16) for vcg in range(8)]
    + [(vcg, 1, 32, 16) for vcg in range(8)]
    + [(2 * g, 2, 48, 8) for g in range(4)]
    + [(4 * g, 4, 56, 4) for g in range(2)]
    + [(4 * g, 4, 60, 2) for g in range(2)]
    + [(0, 8, 62, 1), (0, 8, 63, 1)]
)
# earliest superstep at which each task's decT inputs exist
TASK_AVAIL = ([17] * 8 + [33] * 8 + [49] * 8 + [57] * 4 + [61] * 2
              + [63] * 2 + [64, 65])


def _reorder_w(Wih, Whh):
    """[128, 8*2048]: rounds 0-3 = Wih K-chunks, 4-7 = Whh K-chunks.
    col j*512 + g*128 + x  <-  W[g*512 + 128j + x, 128*ki + p]; g-gate rows x2
    (tanh(z) = 2*sigmoid(2z) - 1 lets one Sigmoid call cover all gates)."""
    out = np.zeros((128, 8 * 2048), np.float32)
    for r in range(8):
        Wsrc = Wih if r < 4 else Whh
        ki = r % 4
        blk = Wsrc[:, 128 * ki:128 * (ki + 1)]          # [2048, 128] (gates, p)
        t_ = blk.reshape(4, 4, 128, 128)                # [g, j, x, p]
        t_ = t_.transpose(3, 1, 0, 2)                   # [p, j, g, x]
        out[:, r * 2048:(r + 1) * 2048] = t_.reshape(128, 2048)
    w5 = out.reshape(128, 8, 4, 4, 128)                 # [p, r, j, g, x]
    w5[:, :, :, 2, :] *= 2.0
    return out


def _build_nc():
    import concourse.bass as bass
    import concourse.bacc as bacc
    import concourse.mybir as mybir
    import concourse.tile as tile

    f32 = mybir.dt.float32
    bf16 = mybir.dt.bfloat16
    f8 = mybir.dt.float8e4 if USE_FP8 else mybir.dt.bfloat16
    AF = mybir.ActivationFunctionType
    OP = mybir.AluOpType

    nc = bacc.Bacc(None, target_bir_lowering=False)
    d = {}
    d["W0"] = nc.dram_tensor("W0", [128, 4 * 2048], f8, kind="ExternalInput")
    d["W1"] = nc.dram_tensor("W1", [128, 8 * 2048], f8, kind="ExternalInput")
    d["Wd"] = nc.dram_tensor("Wd", [128, 4 * VSP], bf16, kind="ExternalInput")
    d["ig0"] = nc.dram_tensor("ig0", [128, T * 128], bf16, kind="ExternalInput")
    d["misc"] = nc.dram_tensor("misc", [128, 448], bf16, kind="ExternalInput")
    d["c0T"] = nc.dram_tensor("c0T", [128, 64], bf16, kind="ExternalInput")
    out_d = nc.dram_tensor("out", [128, T * 256], bf16,
                           kind="ExternalOutput")

    with tile.TileContext(nc) as tc:
        with (
            tc.tile_pool(name="const", bufs=1) as cp,
            tc.tile_pool(name="work", bufs=4) as wp,
            tc.tile_pool(name="psA", bufs=2, space="PSUM") as ppA,
            tc.tile_pool(name="psB", bufs=2, space="PSUM") as ppB,
            tc.tile_pool(name="psP", bufs=3, space="PSUM") as ppP,
        ):
            W0s = cp.tile([128, 4 * 2048], f8, tag="W0s")
            W1s = cp.tile([128, 8 * 2048], f8, tag="W1s")
            Wds = cp.tile([128, 4 * VSP], bf16, tag="Wds")
            ig0s = cp.tile([128, T * 128], bf16, tag="ig0s")
            misc_sb = cp.tile([128, 448], bf16, tag="misc")
            ids = misc_sb[:, 0:128]
            b1s = misc_sb[:, 128:256]
            dec0T = cp.tile([128, (T + 1) * 32], bf16, tag="dec0T")
            decT = cp.tile([128, (T + 1) * 32], bf16, tag="decT")
            c_sb = cp.tile([128, 64], bf16, tag="c_sb")
            stage = cp.tile([128, T * 256], bf16, tag="stage")

            # Step-0 inputs first, then weights in consumption order.
            nc.sync.dma_start(misc_sb[:], d["misc"][:])
            nc.sync.dma_start(c_sb[:], d["c0T"][:])
            nc.sync.dma_start(ig0s[:, 128:512], d["ig0"][:, 128:512])
            for r in range(4):
                nc.sync.dma_start(W0s[:, r * 2048:(r + 1) * 2048],
                                  d["W0"][:, r * 2048:(r + 1) * 2048])
            for r in range(4):
                nc.sync.dma_start(W1s[:, r * 4096:(r + 1) * 4096],
                                  d["W1"][:, r * 4096:(r + 1) * 4096])
            nc.sync.dma_start(ig0s[:, 512:2048], d["ig0"][:, 512:2048])
            for r in range(1, 4):
                nc.sync.dma_start(ig0s[:, r * 2048:(r + 1) * 2048],
                                  d["ig0"][:, r * 2048:(r + 1) * 2048])
            for r in range(4):
                nc.sync.dma_start(Wds[:, r * VSP:(r + 1) * VSP],
                                  d["Wd"][:, r * VSP:(r + 1) * VSP])

            own = [dec0T, decT]
            pools = [ppA, ppB]
            ps_t = [{}, {}]   # layer -> t -> psum tile
            sg_t = [{}, {}]   # layer -> t -> sigmoid output tile
            cn_t = [{}, {}]   # layer -> t -> new-c tile

            def inject(layer, t):
                ps = pools[layer].tile([128, 128], f32, tag=f"ps{layer}",
                                       name=f"ps{layer}_{t}")
                ps_t[layer][t] = ps
                if layer == 0:
                    src = (misc_sb[:, 320:448] if t == 0
                           else ig0s[:, 128 * t:128 * (t + 1)])
                else:
                    src = b1s[:]
                nc.tensor.matmul(ps[:], src, ids[:], start=True, stop=False,
                                 skip_group_check=True)

            def mm_x(s):
                # layer-1 input-side matmuls (dec0 -> gates); off the
                # critical path (dec0T[s+1] is ready before rec1(s) runs).
                ps = ps_t[1][s]
                for k in range(4):
                    hs = dec0T[:, 32 * (s + 1) + 8 * k: 32 * (s + 1) + 8 * k + 8]
                    for j in range(4):
                        for g in range(4):
                            mw = k * 2048 + (4 * j + g) * 128
                            nc.tensor.matmul(
                                ps[:, g * 32 + j * 8: g * 32 + j * 8 + 8],
                                W1s[:, mw: mw + 128],
                                hs, start=False, stop=False,
                                skip_group_check=True)

            def mm_rec(layer, t):
                ps = ps_t[layer][t]
                Wr = W0s if layer == 0 else W1s
                roff = 0 if layer == 0 else 4 * 2048
                if t == 0:
                    src = misc_sb
                    base = 256 + 32 * layer
                else:
                    src = own[layer]
                    base = 32 * t
                for k in range(4):
                    hs = src[:, base + 8 * k: base + 8 * k + 8]
                    for j in range(4):
                        for g in range(4):
                            last = (k == 3 and j == 3 and g == 3)
                            mw = roff + k * 2048 + (4 * j + g) * 128
                            nc.tensor.matmul(
                                ps[:, g * 32 + j * 8: g * 32 + j * 8 + 8],
                                Wr[:, mw: mw + 128],
                                hs, start=False, stop=last,
                                skip_group_check=True)

            def sig(layer, t):
                sg = wp.tile([128, 128], bf16, tag=f"sg{layer}")
                sg_t[layer][t] = sg
                nc.scalar.activation(sg[:], ps_t[layer][t][:], AF.Sigmoid)

            def cupd(layer, t):
                # c = sig(f)*c + sig(i)*tanh(zg); tanh(zg) = 2*sig(2zg)-1 and
                # the 2x is baked into the g-gate weights, so with
                # m2 = (sg'-0.5)*si:  c_new/2 = m2 + sig(f)*(c/2).
                sg = sg_t[layer][t]
                cs = c_sb[:, layer * 32:(layer + 1) * 32]
                m2 = wp.tile([128, 32], bf16, tag=f"m2{layer}")
                nc.vector.scalar_tensor_tensor(m2[:], sg[:, 64:96], 0.5,
                                               sg[:, 0:32],
                                               OP.subtract, OP.mult)
                m1 = wp.tile([128, 32], bf16, tag=f"m1{layer}")
                nc.vector.tensor_mul(m1[:], sg[:, 32:64], cs)
                nc.vector.tensor_add(cs, m2[:], m1[:])

            def ctanh(layer, t):
                cs = c_sb[:, layer * 32:(layer + 1) * 32]
                cn = wp.tile([128, 32], bf16, tag=f"cn{layer}")
                cn_t[layer][t] = cn
                nc.scalar.activation(cn[:], cs, AF.Tanh, scale=2.0)

            def hmul(layer, t):
                sg = sg_t[layer][t]
                nc.vector.tensor_mul(own[layer][:, 32 * (t + 1):32 * (t + 2)],
                                     sg[:, 96:128], cn_t[layer][t][:])

            # ---------- projection ----------
            # stage col = t*256 + vc*8 + b (t-major: tail regions DMA early)
            decv = decT.rearrange("p (s c) -> p s c", c=32)
            stg = stage.rearrange("p (t v b) -> p v t b", t=T, v=NVC, b=8)
            pq = list(TASKS)
            emit_proj_idx = [0]
            copy_q = []     # pending (psP, vcg0, nvcg, t0, nt, vl0, vl1)
            # DMA regions [t0, t1, pieces_needed, pieces_done]
            regions = [[0, 16, 16, 0], [16, 32, 16, 0], [32, 48, 16, 0],
                       [48, 56, 8, 0], [56, 60, 4, 0], [60, 62, 4, 0],
                       [62, 63, 2, 0], [63, 64, 2, 0]]

            def emit_proj(n):
                for _ in range(n):
                    if emit_proj_idx[0] >= len(pq):
                        return
                    i = emit_proj_idx[0]
                    emit_proj_idx[0] += 1
                    vcg0, nvcg, t0, nt = pq[i]
                    w = nt * 8
                    nvc = 4 * nvcg
                    psP = ppP.tile([128, 512], f32, tag="psP",
                                   name=f"psP_{t0}_{vcg0}")
                    nh = 2 if nt > 8 else 1     # split N to bound PE HOL delay
                    for vl in range(nvc):
                        vc = 4 * vcg0 + vl
                        for k in range(4):
                            for h2 in range(nh):
                                s0 = t0 + 1 + (nt // nh) * h2
                                sn = nt // nh
                                nc.tensor.matmul(
                                    psP[:, vl * w + sn * 8 * h2:
                                        vl * w + sn * 8 * (h2 + 1)],
                                    Wds[:, k * VSP + vc * 128:
                                        k * VSP + (vc + 1) * 128],
                                    decv[:, s0: s0 + sn, 8 * k:8 * k + 8],
                                    start=(k == 0 and h2 == 0),
                                    stop=(k == 3 and h2 == nh - 1),
                                    skip_group_check=True)
                    # two ~256-col copy pieces per task, emitted later at
                    # engine-idle points of the chain schedule
                    copy_q.append((psP, vcg0, nvcg, t0, nt, 0, nvc // 2))
                    copy_q.append((psP, vcg0, nvcg, t0, nt, nvc // 2, nvc))

            def emit_copy(n, eng):
                for _ in range(n):
                    if not copy_q:
                        return
                    psP, vcg0, nvcg, t0, nt, vl0, vl1 = copy_q.pop(0)
                    pv4 = psP.rearrange("p (v t b) -> p v t b", t=nt, b=8)
                    src = pv4[:, vl0:vl1, :, :]
                    dst = stg[:, 4 * vcg0 + vl0:4 * vcg0 + vl1, t0:t0 + nt, :]
                    if eng == "act":
                        nc.scalar.activation(dst, src, AF.Copy)
                    else:
                        nc.vector.tensor_copy(dst, src)
                    for reg in regions:
                        if reg[0] <= t0 < reg[1]:
                            reg[3] += 1
                            if reg[3] == reg[2]:
                                nc.sync.dma_start(
                                    out_d[:, reg[0] * 256:reg[1] * 256],
                                    stage[:, reg[0] * 256:reg[1] * 256])

            # ---------- main loop ----------
            inject(0, 0)
            for t in range(T):
                s = t - 1   # layer-1 step handled this superstep
                mm_rec(0, t)
                if s >= 0:
                    mm_x(s)
                    mm_rec(1, s)
                if t + 1 < T:
                    inject(0, t + 1)
                inject(1, t)
                sig(0, t)
                if s >= 0:
                    sig(1, s)
                cupd(0, t)
                ctanh(0, t)
                if s >= 0:
                    cupd(1, s)
                    ctanh(1, s)
                hmul(0, t)
                if s >= 0:
                    hmul(1, s)
                emit_copy(1, "act")
                emit_copy(1, "dve")
                # projection task feed (keeps PE busy in the chain gaps);
                # gated on TASK_AVAIL so queued matmuls never head-of-line
                # block the PE waiting for future decT blocks.
                budget = 2 if t == 63 else 1
                while (budget and emit_proj_idx[0] < len(pq)
                       and TASK_AVAIL[emit_proj_idx[0]] <= t):
                    emit_proj(1)
                    budget -= 1
            # drain layer-1 step T-1
            s = T - 1
            mm_x(s)
            mm_rec(1, s)
            emit_proj(1)            # (62,1): data completes mid-drain
            sig(1, s)
            emit_copy(1, "act")
            emit_copy(1, "dve")
            cupd(1, s)
            ctanh(1, s)
            hmul(1, s)
            emit_proj(len(pq) - emit_proj_idx[0])
            while copy_q:
                emit_copy(1, "dve")
                emit_copy(1, "act")
    nc.finalize()
    return nc


_NC_CACHE = None


def _get_nc():
    global _NC_CACHE
    if _NC_CACHE is None:
        _NC_CACHE = _build_nc()
    return _NC_CACHE


def _host_inputs(input_ids, enc_output, h0, c0, emb, Wih0, Whh0, bih0, bhh0,
                 Wih1, Whh1, bih1, bhh1, W1, b1, W2, b2, w3, b3, Wout, bout):
    f32 = np.float32
    x = np.asarray(emb, f32)[np.asarray(input_ids).astype(np.int64)]  # [B,T,E]

    # Layer-0 input projection on the host (exact), g-gate x2, bias folded in.
    ig0 = x @ np.asarray(Wih0, f32).T + (np.asarray(bih0, f32)
                                         + np.asarray(bhh0, f32))   # [B,T,2048]
    ig0 = ig0.reshape(B, T, 4, 4, 128)          # [b,t,g,j,x]
    ig0[:, :, 2] *= 2.0
    ig0T = ig0.transpose(2, 3, 0, 1, 4).reshape(128, T * 128)  # [(g,j,b),(t,x)]

    b1v = (np.asarray(bih1, f32) + np.asarray(bhh1, f32)).reshape(4, 4, 128)
    b1v = b1v.copy()
    b1v[2] *= 2.0                               # [g,j,x]
    b1T = np.broadcast_to(b1v[:, :, None, :], (4, 4, 8, 128)).reshape(128, 128)

    def h0T(hl):
        return hl.T.reshape(4, 128, 8).transpose(1, 0, 2).reshape(128, 32)

    c0a = (np.asarray(c0, f32) * 0.5).reshape(2, 8, 4, 128)
    c0T = c0a.transpose(3, 0, 2, 1).reshape(128, 64)  # [x, (layer,j,b)]

    # collapsed attention (exact in real arithmetic; see module docstring)
    u = np.asarray(W2, f32).T @ np.asarray(w3, f32)[0]
    ue = np.asarray(W1, f32)[:, :H].T @ u
    sc = np.asarray(enc_output, f32) @ ue                  # [B,S]
    sc = sc - sc.max(-1, keepdims=True)
    a = np.exp(sc)
    a /= a.sum(-1, keepdims=True)
    ctxh = np.einsum('bs,bsh->bh', a, np.asarray(enc_output, f32))  # [B,H]

    Wo_full = np.asarray(Wout, f32)                        # [V, 2H]
    bo_full = np.asarray(bout, f32)
    # t-independent half of the projection, added on the host
    ctxadd = ctxh @ Wo_full[:, H:].T + bo_full             # [B, V]

    Wrec = _reorder_w(np.asarray(Wih0, f32), np.asarray(Whh0, f32))
    misc = np.concatenate([np.eye(128, dtype=f32), b1T,
                           h0T(np.asarray(h0, f32)[0]),
                           h0T(np.asarray(h0, f32)[1]),
                           ig0T[:, 0:128]], axis=1)
    base = {
        "W0": np.ascontiguousarray(Wrec[:, 4 * 2048:]).astype(_F8 if USE_FP8 else _BF16),
        "W1": _reorder_w(np.asarray(Wih1, f32),
                         np.asarray(Whh1, f32)).astype(_F8 if USE_FP8 else _BF16),
        "ig0": ig0T.astype(_BF16),
        "misc": misc.astype(_BF16),
        "c0T": c0T.astype(_BF16),
    }
    maps = []
    for k in range(NCORES):
        lo = k * VS
        sh = np.zeros((VSP, H), f32)
        n = min(VSP, V - lo)
        sh[:n] = Wo_full[lo:lo + n, :H]
        t_ = sh.reshape(NVC, 128, 4, 128).transpose(3, 2, 0, 1)  # [p,k,vc,m]
        m = dict(base)
        m["Wd"] = np.ascontiguousarray(t_.reshape(128, 4 * VSP)).astype(_BF16)
        maps.append(m)
    return maps, ctxadd


def kernel(**inputs):
    from concourse.bass_utils import run_bass_kernel_spmd
    nc = _get_nc()
    maps, ctxadd = _host_inputs(**inputs)
    res = run_bass_kernel_spmd(nc, maps, list(range(NCORES))).results
    full = np.zeros((B, T, V), np.float32)
    for k in range(NCORES):
        o = np.asarray(res[k]["out"], np.float32)   # [128, T*256]
        o = o.reshape(128, T, NVC, 8)               # [x, t, vc, b]
        o = o.transpose(3, 1, 2, 0).reshape(B, T, VSP)
        n = min(VS, V - k * VS)
        full[:, :, k * VS:k * VS + n] = o[:, :, :n]
    full += ctxadd[:, None, :]
    return full


# revision 24
# speedup vs baseline: 1.1154x; 1.0034x over previous
"""AttnDecoder kernel for 8 trn2 NeuronCores — latency-optimized chain design.

Math notes (exact in real arithmetic):
 - The reference's additive attention has no nonlinearity between W1/W2/w3, so
   softmax over s cancels every t-dependent term: attn (and ctx) are
   t-independent. ctx[b] is computed on the host.
 - logits = dec @ Wout[:, :H].T + (ctx @ Wout[:, H:].T + bout); the second
   term is t-independent and is added on the host.
 - Device work: the 2-layer LSTM recurrence (replicated on all 8 cores) and
   the dec-half of the vocab projection (vocab-sharded, 4096 padded cols per
   core).

Layout: everything is "output-transposed" — matmul outputs keep hidden/vocab
dims on partitions and the batch (8) on the free dim. Gate pre-activations for
step t live in one PSUM tile [128, 128] with col = gate*32 + j*8 + b
(j = h-dim block); one Sigmoid covers all four gates (tanh(z) = 2*sig(2z)-1
with the 2x baked into the g-gate weights/inputs).

Schedule: the two layers' recurrences are independent chains (layer 1 step
s = t-1 only needs superstep t-1 outputs), interleaved per-engine in data
arrival order so neither blocks the other on the in-order queues:
  PE : rec0(t) | x1(s) | rec1(s) | inject(t+1) | proj matmuls
  Act: sig0(t) | sig1(s) | tanh0(t) | tanh1(s)
  DVE: c-update0(t) | c-update1(s) | hmul0(t) | hmul1(s)
  Pool: projection PSUM->SBUF copies (keeps Act/DVE free for the chain)
Recurrent weights are fp8-e4m3 (halves the weight-load DMA that gates the
layer-1 chain start); activations stay bf16.
"""

import numpy as np
import ml_dtypes

B, T, S = 8, 64, 128
V, E, H = 32000, 512, 512
NCORES = 8
VS = V // NCORES   # 4000 real vocab cols per core
VSP = 4096         # padded to 32 chunks of 128
NVC = VSP // 128   # 32 vocab chunks

_BF16 = ml_dtypes.bfloat16
_F8 = ml_dtypes.float8_e4m3fn
USE_FP8 = True

# Projection task list, in emission order: (vcg0, nvcg, t0, nt).
# Each task fills one PSUM tile with logits for vocab chunks
# [4*vcg0, 4*(vcg0+nvcg)) and decoder steps [t0, t0+nt), laid out
# col = vc_local*(nt*8) + tl*8 + b, then DMAs it straight to DRAM slot
# task_idx*512 (f32). Host unscrambles. Tail tasks pack several vocab
# groups so the end-of-program DMA burst stays short.
TASKS = (
    [(vcg, 1, 0, 16) for vcg in range(8)]
    + [(vcg, 1, 16, 16) for vcg in range(8)]
    + [(vcg, 1, 32, 16) for vcg in range(8)]
    + [(2 * g, 2, 48, 8) for g in range(4)]
    + [(4 * g, 4, 56, 4) for g in range(2)]
    + [(4 * g, 4, 60, 2) for g in range(2)]
    + [(0, 8, 62, 1), (0, 8, 63, 1)]
)
# earliest superstep at which each task's decT inputs exist
TASK_AVAIL = ([17] * 8 + [33] * 8 + [49] * 8 + [57] * 4 + [61] * 2
              + [63] * 2 + [64, 65])


def _reorder_w(Wih, Whh):
    """[128, 8*2048]: rounds 0-3 = Wih K-chunks, 4-7 = Whh K-chunks.
    col j*512 + g*128 + x  <-  W[g*512 + 128j + x, 128*ki + p]; g-gate rows x2
    (tanh(z) = 2*sigmoid(2z) - 1 lets one Sigmoid call cover all gates)."""
    out = np.zeros((128, 8 * 2048), np.float32)
    for r in range(8):
        Wsrc = Wih if r < 4 else Whh
        ki = r % 4
        blk = Wsrc[:, 128 * ki:128 * (ki + 1)]          # [2048, 128] (gates, p)
        t_ = blk.reshape(4, 4, 128, 128)                # [g, j, x, p]
        t_ = t_.transpose(3, 1, 0, 2)                   # [p, j, g, x]
        out[:, r * 2048:(r + 1) * 2048] = t_.reshape(128, 2048)
    w5 = out.reshape(128, 8, 4, 4, 128)                 # [p, r, j, g, x]
    w5[:, :, :, 2, :] *= 2.0
    return out


def _build_nc():
    import concourse.bass as bass
    import concourse.bacc as bacc
    import concourse.mybir as mybir
    import concourse.tile as tile

    f32 = mybir.dt.float32
    bf16 = mybir.dt.bfloat16
    f8 = mybir.dt.float8e4 if USE_FP8 else mybir.dt.bfloat16
    AF = mybir.ActivationFunctionType
    OP = mybir.AluOpType

    nc = bacc.Bacc(None, target_bir_lowering=False)
    d = {}
    d["W0"] = nc.dram_tensor("W0", [128, 4 * 2048], f8, kind="ExternalInput")
    d["W1"] = nc.dram_tensor("W1", [128, 8 * 2048], f8, kind="ExternalInput")
    d["Wd"] = nc.dram_tensor("Wd", [128, 4 * VSP], bf16, kind="ExternalInput")
    d["ig0"] = nc.dram_tensor("ig0", [128, T * 128], bf16, kind="ExternalInput")
    d["misc"] = nc.dram_tensor("misc", [128, 448], bf16, kind="ExternalInput")
    d["c0T"] = nc.dram_tensor("c0T", [128, 64], bf16, kind="ExternalInput")
    out_d = nc.dram_tensor("out", [128, T * 256], bf16,
                           kind="ExternalOutput")

    with tile.TileContext(nc) as tc:
        with (
            tc.tile_pool(name="const", bufs=1) as cp,
            tc.tile_pool(name="work", bufs=4) as wp,
            tc.tile_pool(name="psA", bufs=2, space="PSUM") as ppA,
            tc.tile_pool(name="psB", bufs=2, space="PSUM") as ppB,
            tc.tile_pool(name="psP", bufs=3, space="PSUM") as ppP,
        ):
            W0s = cp.tile([128, 4 * 2048], f8, tag="W0s")
            W1s = cp.tile([128, 8 * 2048], f8, tag="W1s")
            Wds = cp.tile([128, 4 * VSP], bf16, tag="Wds")
            ig0s = cp.tile([128, T * 128], bf16, tag="ig0s")
            misc_sb = cp.tile([128, 448], bf16, tag="misc")
            ids = misc_sb[:, 0:128]
            b1s = misc_sb[:, 128:256]
            dec0T = cp.tile([128, (T + 1) * 32], bf16, tag="dec0T")
            decT = cp.tile([128, (T + 1) * 32], bf16, tag="decT")
            c_sb = cp.tile([128, 64], bf16, tag="c_sb")
            stage = cp.tile([128, T * 256], bf16, tag="stage")

            # Step-0 inputs first, then weights in consumption order.
            nc.sync.dma_start(misc_sb[:], d["misc"][:])
            nc.sync.dma_start(c_sb[:], d["c0T"][:])
            nc.sync.dma_start(ig0s[:, 128:512], d["ig0"][:, 128:512])
            for r in range(4):
                nc.sync.dma_start(W0s[:, r * 2048:(r + 1) * 2048],
                                  d["W0"][:, r * 2048:(r + 1) * 2048])
            for r in range(4):
                nc.sync.dma_start(W1s[:, r * 4096:(r + 1) * 4096],
                                  d["W1"][:, r * 4096:(r + 1) * 4096])
            nc.sync.dma_start(ig0s[:, 512:2048], d["ig0"][:, 512:2048])
            for r in range(1, 4):
                nc.sync.dma_start(ig0s[:, r * 2048:(r + 1) * 2048],
                                  d["ig0"][:, r * 2048:(r + 1) * 2048])
            for r in range(4):
                nc.sync.dma_start(Wds[:, r * VSP:(r + 1) * VSP],
                                  d["Wd"][:, r * VSP:(r + 1) * VSP])

            own = [dec0T, decT]
            pools = [ppA, ppB]
            ps_t = [{}, {}]   # layer -> t -> psum tile
            sg_t = [{}, {}]   # layer -> t -> sigmoid output tile
            cn_t = [{}, {}]   # layer -> t -> new-c tile

            def inject(layer, t):
                ps = pools[layer].tile([128, 128], f32, tag=f"ps{layer}",
                                       name=f"ps{layer}_{t}")
                ps_t[layer][t] = ps
                if layer == 0:
                    src = (misc_sb[:, 320:448] if t == 0
                           else ig0s[:, 128 * t:128 * (t + 1)])
                else:
                    src = b1s[:]
                nc.tensor.matmul(ps[:], src, ids[:], start=True, stop=False,
                                 skip_group_check=True)

            def mm_x(s):
                # layer-1 input-side matmuls (dec0 -> gates); off the
                # critical path (dec0T[s+1] is ready before rec1(s) runs).
                ps = ps_t[1][s]
                for k in range(4):
                    hs = dec0T[:, 32 * (s + 1) + 8 * k: 32 * (s + 1) + 8 * k + 8]
                    for j in range(4):
                        for g in range(4):
                            mw = k * 2048 + (4 * j + g) * 128
                            nc.tensor.matmul(
                                ps[:, g * 32 + j * 8: g * 32 + j * 8 + 8],
                                W1s[:, mw: mw + 128],
                                hs, start=False, stop=False,
                                skip_group_check=True)

            def mm_rec(layer, t):
                ps = ps_t[layer][t]
                Wr = W0s if layer == 0 else W1s
                roff = 0 if layer == 0 else 4 * 2048
                if t == 0:
                    src = misc_sb
                    base = 256 + 32 * layer
                else:
                    src = own[layer]
                    base = 32 * t
                for k in range(4):
                    hs = src[:, base + 8 * k: base + 8 * k + 8]
                    for j in range(4):
                        for g in range(4):
                            last = (k == 3 and j == 3 and g == 3)
                            mw = roff + k * 2048 + (4 * j + g) * 128
                            nc.tensor.matmul(
                                ps[:, g * 32 + j * 8: g * 32 + j * 8 + 8],
                                Wr[:, mw: mw + 128],
                                hs, start=False, stop=last,
                                skip_group_check=True)

            def sig(layer, t):
                sg = wp.tile([128, 128], bf16, tag=f"sg{layer}")
                sg_t[layer][t] = sg
                nc.scalar.activation(sg[:], ps_t[layer][t][:], AF.Sigmoid)

            def cupd(layer, t):
                # c = sig(f)*c + sig(i)*tanh(zg); tanh(zg) = 2*sig(2zg)-1 and
                # the 2x is baked into the g-gate weights, so with
                # m2 = (sg'-0.5)*si:  c_new/2 = m2 + sig(f)*(c/2).
                sg = sg_t[layer][t]
                cs = c_sb[:, layer * 32:(layer + 1) * 32]
                m2 = wp.tile([128, 32], bf16, tag=f"m2{layer}")
                nc.vector.scalar_tensor_tensor(m2[:], sg[:, 64:96], 0.5,
                                               sg[:, 0:32],
                                               OP.subtract, OP.mult)
                m1 = wp.tile([128, 32], bf16, tag=f"m1{layer}")
                nc.vector.tensor_mul(m1[:], sg[:, 32:64], cs)
                nc.vector.tensor_add(cs, m2[:], m1[:])

            def ctanh(layer, t):
                cs = c_sb[:, layer * 32:(layer + 1) * 32]
                cn = wp.tile([128, 32], bf16, tag=f"cn{layer}")
                cn_t[layer][t] = cn
                nc.scalar.activation(cn[:], cs, AF.Tanh, scale=2.0)

            def hmul(layer, t):
                # layer 1's h-mul lands late in the superstep; on DVE it
                # head-of-line blocks layer 0's c-update, so it runs on the
                # otherwise-idle Pool engine (all-SBUF op).
                sg = sg_t[layer][t]
                eng = nc.vector if layer == 0 else nc.gpsimd
                eng.tensor_mul(own[layer][:, 32 * (t + 1):32 * (t + 2)],
                               sg[:, 96:128], cn_t[layer][t][:])

            # ---------- projection ----------
            # stage col = t*256 + vc*8 + b (t-major: tail regions DMA early)
            decv = decT.rearrange("p (s c) -> p s c", c=32)
            stg = stage.rearrange("p (t v b) -> p v t b", t=T, v=NVC, b=8)
            pq = list(TASKS)
            emit_proj_idx = [0]
            copy_q = []     # pending (psP, vcg0, nvcg, t0, nt, vl0, vl1)
            # DMA regions [t0, t1, pieces_needed, pieces_done]
            regions = [[0, 16, 16, 0], [16, 32, 16, 0], [32, 48, 16, 0],
                       [48, 56, 8, 0], [56, 60, 4, 0], [60, 62, 4, 0],
                       [62, 63, 2, 0], [63, 64, 2, 0]]

            def emit_proj(n):
                for _ in range(n):
                    if emit_proj_idx[0] >= len(pq):
                        return
                    i = emit_proj_idx[0]
                    emit_proj_idx[0] += 1
                    vcg0, nvcg, t0, nt = pq[i]
                    w = nt * 8
                    nvc = 4 * nvcg
                    psP = ppP.tile([128, 512], f32, tag="psP",
                                   name=f"psP_{t0}_{vcg0}")
                    nh = 2 if nt > 8 else 1     # split N to bound PE HOL delay
                    for vl in range(nvc):
                        vc = 4 * vcg0 + vl
                        for k in range(4):
                            for h2 in range(nh):
                                s0 = t0 + 1 + (nt // nh) * h2
                                sn = nt // nh
                                nc.tensor.matmul(
                                    psP[:, vl * w + sn * 8 * h2:
                                        vl * w + sn * 8 * (h2 + 1)],
                                    Wds[:, k * VSP + vc * 128:
                                        k * VSP + (vc + 1) * 128],
                                    decv[:, s0: s0 + sn, 8 * k:8 * k + 8],
                                    start=(k == 0 and h2 == 0),
                                    stop=(k == 3 and h2 == nh - 1),
                                    skip_group_check=True)
                    # two ~256-col copy pieces per task, emitted later at
                    # engine-idle points of the chain schedule
                    copy_q.append((psP, vcg0, nvcg, t0, nt, 0, nvc // 2))
                    copy_q.append((psP, vcg0, nvcg, t0, nt, nvc // 2, nvc))

            def emit_copy(n, eng):
                for _ in range(n):
                    if not copy_q:
                        return
                    psP, vcg0, nvcg, t0, nt, vl0, vl1 = copy_q.pop(0)
                    pv4 = psP.rearrange("p (v t b) -> p v t b", t=nt, b=8)
                    src = pv4[:, vl0:vl1, :, :]
                    dst = stg[:, 4 * vcg0 + vl0:4 * vcg0 + vl1, t0:t0 + nt, :]
                    if eng == "act":
                        nc.scalar.activation(dst, src, AF.Copy)
                    else:
                        nc.vector.tensor_copy(dst, src)
                    for reg in regions:
                        if reg[0] <= t0 < reg[1]:
                            reg[3] += 1
                            if reg[3] == reg[2]:
                                nc.sync.dma_start(
                                    out_d[:, reg[0] * 256:reg[1] * 256],
                                    stage[:, reg[0] * 256:reg[1] * 256])

            # ---------- main loop ----------
            inject(0, 0)
            for t in range(T):
                s = t - 1   # layer-1 step handled this superstep
                mm_rec(0, t)
                if s >= 0:
                    mm_x(s)
                    mm_rec(1, s)
                if t + 1 < T:
                    inject(0, t + 1)
                inject(1, t)
                sig(0, t)
                if s >= 0:
                    sig(1, s)
                cupd(0, t)
                ctanh(0, t)
                if s >= 0:
                    cupd(1, s)
                    ctanh(1, s)
                hmul(0, t)
                if s >= 0:
                    hmul(1, s)
                emit_copy(1, "act")
                emit_copy(1, "dve")
                # projection task feed (keeps PE busy in the chain gaps);
                # gated on TASK_AVAIL so queued matmuls never head-of-line
                # block the PE waiting for future decT blocks.
                budget = 2 if t == 63 else 1
                while (budget and emit_proj_idx[0] < len(pq)
                       and TASK_AVAIL[emit_proj_idx[0]] <= t):
                    emit_proj(1)
                    budget -= 1
            # drain layer-1 step T-1
            s = T - 1
            mm_x(s)
            mm_rec(1, s)
            emit_proj(1)            # (62,1): data completes mid-drain
            sig(1, s)
            emit_copy(1, "act")
            emit_copy(1, "dve")
            cupd(1, s)
            ctanh(1, s)
            hmul(1, s)
            emit_proj(len(pq) - emit_proj_idx[0])
            while copy_q:
                emit_copy(1, "dve")
                emit_copy(1, "act")
    nc.finalize()
    return nc


_NC_CACHE = None


def _get_nc():
    global _NC_CACHE
    if _NC_CACHE is None:
        _NC_CACHE = _build_nc()
    return _NC_CACHE


def _host_inputs(input_ids, enc_output, h0, c0, emb, Wih0, Whh0, bih0, bhh0,
                 Wih1, Whh1, bih1, bhh1, W1, b1, W2, b2, w3, b3, Wout, bout):
    f32 = np.float32
    x = np.asarray(emb, f32)[np.asarray(input_ids).astype(np.int64)]  # [B,T,E]

    # Layer-0 input projection on the host (exact), g-gate x2, bias folded in.
    ig0 = x @ np.asarray(Wih0, f32).T + (np.asarray(bih0, f32)
                                         + np.asarray(bhh0, f32))   # [B,T,2048]
    ig0 = ig0.reshape(B, T, 4, 4, 128)          # [b,t,g,j,x]
    ig0[:, :, 2] *= 2.0
    ig0T = ig0.transpose(2, 3, 0, 1, 4).reshape(128, T * 128)  # [(g,j,b),(t,x)]

    b1v = (np.asarray(bih1, f32) + np.asarray(bhh1, f32)).reshape(4, 4, 128)
    b1v = b1v.copy()
    b1v[2] *= 2.0                               # [g,j,x]
    b1T = np.broadcast_to(b1v[:, :, None, :], (4, 4, 8, 128)).reshape(128, 128)

    def h0T(hl):
        return hl.T.reshape(4, 128, 8).transpose(1, 0, 2).reshape(128, 32)

    c0a = (np.asarray(c0, f32) * 0.5).reshape(2, 8, 4, 128)
    c0T = c0a.transpose(3, 0, 2, 1).reshape(128, 64)  # [x, (layer,j,b)]

    # collapsed attention (exact in real arithmetic; see module docstring)
    u = np.asarray(W2, f32).T @ np.asarray(w3, f32)[0]
    ue = np.asarray(W1, f32)[:, :H].T @ u
    sc = np.asarray(enc_output, f32) @ ue                  # [B,S]
    sc = sc - sc.max(-1, keepdims=True)
    a = np.exp(sc)
    a /= a.sum(-1, keepdims=True)
    ctxh = np.einsum('bs,bsh->bh', a, np.asarray(enc_output, f32))  # [B,H]

    Wo_full = np.asarray(Wout, f32)                        # [V, 2H]
    bo_full = np.asarray(bout, f32)
    # t-independent half of the projection, added on the host
    ctxadd = ctxh @ Wo_full[:, H:].T + bo_full             # [B, V]

    Wrec = _reorder_w(np.asarray(Wih0, f32), np.asarray(Whh0, f32))
    misc = np.concatenate([np.eye(128, dtype=f32), b1T,
                           h0T(np.asarray(h0, f32)[0]),
                           h0T(np.asarray(h0, f32)[1]),
                           ig0T[:, 0:128]], axis=1)
    base = {
        "W0": np.ascontiguousarray(Wrec[:, 4 * 2048:]).astype(_F8 if USE_FP8 else _BF16),
        "W1": _reorder_w(np.asarray(Wih1, f32),
                         np.asarray(Whh1, f32)).astype(_F8 if USE_FP8 else _BF16),
        "ig0": ig0T.astype(_BF16),
        "misc": misc.astype(_BF16),
        "c0T": c0T.astype(_BF16),
    }
    maps = []
    for k in range(NCORES):
        lo = k * VS
        sh = np.zeros((VSP, H), f32)
        n = min(VSP, V - lo)
        sh[:n] = Wo_full[lo:lo + n, :H]
        t_ = sh.reshape(NVC, 128, 4, 128).transpose(3, 2, 0, 1)  # [p,k,vc,m]
        m = dict(base)
        m["Wd"] = np.ascontiguousarray(t_.reshape(128, 4 * VSP)).astype(_BF16)
        maps.append(m)
    return maps, ctxadd


def kernel(**inputs):
    from concourse.bass_utils import run_bass_kernel_spmd
    nc = _get_nc()
    maps, ctxadd = _host_inputs(**inputs)
    res = run_bass_kernel_spmd(nc, maps, list(range(NCORES))).results
    full = np.zeros((B, T, V), np.float32)
    for k in range(NCORES):
        o = np.asarray(res[k]["out"], np.float32)   # [128, T*256]
        o = o.reshape(128, T, NVC, 8)               # [x, t, vc, b]
        o = o.transpose(3, 1, 2, 0).reshape(B, T, VSP)
        n = min(VS, V - k * VS)
        full[:, :, k * VS:k * VS + n] = o[:, :, :n]
    full += ctxadd[:, None, :]
    return full


# revision 27
# speedup vs baseline: 1.1232x; 1.0070x over previous
"""AttnDecoder kernel for 8 trn2 NeuronCores — latency-optimized chain design.

Math notes (exact in real arithmetic):
 - The reference's additive attention has no nonlinearity between W1/W2/w3, so
   softmax over s cancels every t-dependent term: attn (and ctx) are
   t-independent. ctx[b] is computed on the host.
 - logits = dec @ Wout[:, :H].T + (ctx @ Wout[:, H:].T + bout); the second
   term is t-independent and is added on the host.
 - Device work: the 2-layer LSTM recurrence (replicated on all 8 cores) and
   the dec-half of the vocab projection (vocab-sharded, 4096 padded cols per
   core).

Layout: everything is "output-transposed" — matmul outputs keep hidden/vocab
dims on partitions and the batch (8) on the free dim. Gate pre-activations for
step t live in one PSUM tile [128, 128] with col = gate*32 + j*8 + b
(j = h-dim block); one Sigmoid covers all four gates (tanh(z) = 2*sig(2z)-1
with the 2x baked into the g-gate weights/inputs).

Schedule: the two layers' recurrences are independent chains (layer 1 step
s = t-1 only needs superstep t-1 outputs), interleaved per-engine in data
arrival order so neither blocks the other on the in-order queues:
  PE : rec0(t) | x1(s) | rec1(s) | inject(t+1) | proj matmuls
  Act: sig0(t) | sig1(s) | tanh0(t) | tanh1(s)
  DVE: c-update0(t) | c-update1(s) | hmul0(t) | hmul1(s)
  Pool: projection PSUM->SBUF copies (keeps Act/DVE free for the chain)
Recurrent weights are fp8-e4m3 (halves the weight-load DMA that gates the
layer-1 chain start); activations stay bf16.
"""

import numpy as np
import ml_dtypes

B, T, S = 8, 64, 128
V, E, H = 32000, 512, 512
NCORES = 8
VS = V // NCORES   # 4000 real vocab cols per core
VSP = 4096         # padded to 32 chunks of 128
NVC = VSP // 128   # 32 vocab chunks

_BF16 = ml_dtypes.bfloat16
_F8 = ml_dtypes.float8_e4m3fn
USE_FP8 = True

# Projection task list, in emission order: (vcg0, nvcg, t0, nt).
# Each task fills one PSUM tile with logits for vocab chunks
# [4*vcg0, 4*(vcg0+nvcg)) and decoder steps [t0, t0+nt), laid out
# col = vc_local*(nt*8) + tl*8 + b, then DMAs it straight to DRAM slot
# task_idx*512 (f32). Host unscrambles. Tail tasks pack several vocab
# groups so the end-of-program DMA burst stays short.
TASKS = (
    [(vcg, 1, 0, 16) for vcg in range(8)]
    + [(vcg, 1, 16, 16) for vcg in range(8)]
    + [(vcg, 1, 32, 16) for vcg in range(8)]
    + [(2 * g, 2, 48, 8) for g in range(4)]
    + [(4 * g, 4, 56, 4) for g in range(2)]
    + [(4 * g, 4, 60, 2) for g in range(2)]
    + [(0, 8, 62, 1), (0, 8, 63, 1)]
)
# earliest superstep at which each task's decT inputs exist
TASK_AVAIL = ([17] * 8 + [33] * 8 + [49] * 8 + [57] * 4 + [61] * 2
              + [63] * 2 + [64, 65])


def _reorder_w(Wih, Whh):
    """[128, 8*2048]: rounds 0-3 = Wih K-chunks, 4-7 = Whh K-chunks.
    col j*512 + g*128 + x  <-  W[g*512 + 128j + x, 128*ki + p]; g-gate rows x2
    (tanh(z) = 2*sigmoid(2z) - 1 lets one Sigmoid call cover all gates)."""
    out = np.zeros((128, 8 * 2048), np.float32)
    for r in range(8):
        Wsrc = Wih if r < 4 else Whh
        ki = r % 4
        blk = Wsrc[:, 128 * ki:128 * (ki + 1)]          # [2048, 128] (gates, p)
        t_ = blk.reshape(4, 4, 128, 128)                # [g, j, x, p]
        t_ = t_.transpose(3, 1, 0, 2)                   # [p, j, g, x]
        out[:, r * 2048:(r + 1) * 2048] = t_.reshape(128, 2048)
    w5 = out.reshape(128, 8, 4, 4, 128)                 # [p, r, j, g, x]
    w5[:, :, :, 2, :] *= 2.0
    return out


def _build_nc():
    import concourse.bass as bass
    import concourse.bacc as bacc
    import concourse.mybir as mybir
    import concourse.tile as tile

    f32 = mybir.dt.float32
    bf16 = mybir.dt.bfloat16
    f8 = mybir.dt.float8e4 if USE_FP8 else mybir.dt.bfloat16
    AF = mybir.ActivationFunctionType
    OP = mybir.AluOpType

    nc = bacc.Bacc(None, target_bir_lowering=False)
    d = {}
    d["W0"] = nc.dram_tensor("W0", [128, 4 * 2048], f8, kind="ExternalInput")
    d["W1"] = nc.dram_tensor("W1", [128, 8 * 2048], f8, kind="ExternalInput")
    d["Wd"] = nc.dram_tensor("Wd", [128, 4 * VSP], bf16, kind="ExternalInput")
    d["ig0"] = nc.dram_tensor("ig0", [128, T * 128], bf16, kind="ExternalInput")
    d["misc"] = nc.dram_tensor("misc", [128, 512], bf16, kind="ExternalInput")
    out_d = nc.dram_tensor("out", [128, T * 256], bf16,
                           kind="ExternalOutput")

    with tile.TileContext(nc) as tc:
        with (
            tc.tile_pool(name="const", bufs=1) as cp,
            tc.tile_pool(name="work", bufs=4) as wp,
            tc.tile_pool(name="psA", bufs=2, space="PSUM") as ppA,
            tc.tile_pool(name="psB", bufs=2, space="PSUM") as ppB,
            tc.tile_pool(name="psP", bufs=3, space="PSUM") as ppP,
        ):
            W0s = cp.tile([128, 4 * 2048], f8, tag="W0s")
            W1s = cp.tile([128, 8 * 2048], f8, tag="W1s")
            Wds = cp.tile([128, 4 * VSP], bf16, tag="Wds")
            ig0s = cp.tile([128, T * 128], bf16, tag="ig0s")
            misc_sb = cp.tile([128, 512], bf16, tag="misc")
            ids = misc_sb[:, 0:128]
            b1s = misc_sb[:, 128:256]
            c_sb = misc_sb[:, 448:512]
            dec0T = cp.tile([128, (T + 1) * 32], bf16, tag="dec0T")
            decT = cp.tile([128, (T + 1) * 32], bf16, tag="decT")
            stage = cp.tile([128, T * 256], bf16, tag="stage")

            # Step-0 inputs first, then weights in consumption order; few
            # big DMAs (each dma_start costs ~625ns of serialized HWDGE).
            nc.sync.dma_start(misc_sb[:], d["misc"][:])
            nc.sync.dma_start(W0s[:], d["W0"][:])
            nc.sync.dma_start(ig0s[:, 128:512], d["ig0"][:, 128:512])
            for r in range(2):
                nc.sync.dma_start(W1s[:, r * 8192:(r + 1) * 8192],
                                  d["W1"][:, r * 8192:(r + 1) * 8192])
            nc.sync.dma_start(ig0s[:, 512:2048], d["ig0"][:, 512:2048])
            for r in range(1, 4):
                nc.sync.dma_start(ig0s[:, r * 2048:(r + 1) * 2048],
                                  d["ig0"][:, r * 2048:(r + 1) * 2048])
            for r in range(4):
                nc.sync.dma_start(Wds[:, r * VSP:(r + 1) * VSP],
                                  d["Wd"][:, r * VSP:(r + 1) * VSP])

            own = [dec0T, decT]
            pools = [ppA, ppB]
            ps_t = [{}, {}]   # layer -> t -> psum tile
            sg_t = [{}, {}]   # layer -> t -> sigmoid output tile
            cn_t = [{}, {}]   # layer -> t -> new-c tile

            def inject(layer, t):
                ps = pools[layer].tile([128, 128], f32, tag=f"ps{layer}",
                                       name=f"ps{layer}_{t}")
                ps_t[layer][t] = ps
                if layer == 0:
                    src = (misc_sb[:, 320:448] if t == 0
                           else ig0s[:, 128 * t:128 * (t + 1)])
                else:
                    src = b1s[:]
                nc.tensor.matmul(ps[:], src, ids[:], start=True, stop=False,
                                 skip_group_check=True)

            def mm_x(s):
                # layer-1 input-side matmuls (dec0 -> gates); off the
                # critical path (dec0T[s+1] is ready before rec1(s) runs).
                ps = ps_t[1][s]
                for k in range(4):
                    hs = dec0T[:, 32 * (s + 1) + 8 * k: 32 * (s + 1) + 8 * k + 8]
                    for j in range(4):
                        for g in range(4):
                            mw = k * 2048 + (4 * j + g) * 128
                            nc.tensor.matmul(
                                ps[:, g * 32 + j * 8: g * 32 + j * 8 + 8],
                                W1s[:, mw: mw + 128],
                                hs, start=False, stop=False,
                                skip_group_check=True)

            def mm_rec(layer, t):
                ps = ps_t[layer][t]
                Wr = W0s if layer == 0 else W1s
                roff = 0 if layer == 0 else 4 * 2048
                if t == 0:
                    src = misc_sb
                    base = 256 + 32 * layer
                else:
                    src = own[layer]
                    base = 32 * t
                for k in range(4):
                    hs = src[:, base + 8 * k: base + 8 * k + 8]
                    for j in range(4):
                        for g in range(4):
                            last = (k == 3 and j == 3 and g == 3)
                            mw = roff + k * 2048 + (4 * j + g) * 128
                            nc.tensor.matmul(
                                ps[:, g * 32 + j * 8: g * 32 + j * 8 + 8],
                                Wr[:, mw: mw + 128],
                                hs, start=False, stop=last,
                                skip_group_check=True)

            def sig(layer, t):
                sg = wp.tile([128, 128], bf16, tag=f"sg{layer}")
                sg_t[layer][t] = sg
                nc.scalar.activation(sg[:], ps_t[layer][t][:], AF.Sigmoid)

            def cupd(layer, t):
                # c = sig(f)*c + sig(i)*tanh(zg); tanh(zg) = 2*sig(2zg)-1 and
                # the 2x is baked into the g-gate weights, so with
                # m2 = (sg'-0.5)*si:  c_new/2 = m2 + sig(f)*(c/2).
                sg = sg_t[layer][t]
                cs = c_sb[:, layer * 32:(layer + 1) * 32]
                m2 = wp.tile([128, 32], bf16, tag=f"m2{layer}")
                nc.vector.scalar_tensor_tensor(m2[:], sg[:, 64:96], 0.5,
                                               sg[:, 0:32],
                                               OP.subtract, OP.mult)
                m1 = wp.tile([128, 32], bf16, tag=f"m1{layer}")
                nc.vector.tensor_mul(m1[:], sg[:, 32:64], cs)
                nc.vector.tensor_add(cs, m2[:], m1[:])

            def ctanh(layer, t):
                cs = c_sb[:, layer * 32:(layer + 1) * 32]
                cn = wp.tile([128, 32], bf16, tag=f"cn{layer}")
                cn_t[layer][t] = cn
                nc.scalar.activation(cn[:], cs, AF.Tanh, scale=2.0)

            def hmul(layer, t):
                # layer 1's h-mul lands late in the superstep; on DVE it
                # head-of-line blocks layer 0's c-update, so it runs on the
                # otherwise-idle Pool engine (all-SBUF op).
                sg = sg_t[layer][t]
                eng = nc.vector if layer == 0 else nc.gpsimd
                eng.tensor_mul(own[layer][:, 32 * (t + 1):32 * (t + 2)],
                               sg[:, 96:128], cn_t[layer][t][:])

            # ---------- projection ----------
            # stage col = t*256 + vc*8 + b (t-major: tail regions DMA early)
            decv = decT.rearrange("p (s c) -> p s c", c=32)
            stg = stage.rearrange("p (t v b) -> p v t b", t=T, v=NVC, b=8)
            pq = list(TASKS)
            emit_proj_idx = [0]
            copy_q = []     # pending (psP, vcg0, nvcg, t0, nt, vl0, vl1)
            # DMA regions [t0, t1, pieces_needed, pieces_done]
            regions = [[0, 16, 16, 0], [16, 32, 16, 0], [32, 48, 16, 0],
                       [48, 56, 8, 0], [56, 60, 4, 0], [60, 62, 4, 0],
                       [62, 63, 2, 0], [63, 64, 2, 0]]

            def emit_proj(n):
                for _ in range(n):
                    if emit_proj_idx[0] >= len(pq):
                        return
                    i = emit_proj_idx[0]
                    emit_proj_idx[0] += 1
                    vcg0, nvcg, t0, nt = pq[i]
                    w = nt * 8
                    nvc = 4 * nvcg
                    psP = ppP.tile([128, 512], f32, tag="psP",
                                   name=f"psP_{t0}_{vcg0}")
                    nh = 2 if nt > 8 else 1     # split N to bound PE HOL delay
                    for vl in range(nvc):
                        vc = 4 * vcg0 + vl
                        for k in range(4):
                            for h2 in range(nh):
                                s0 = t0 + 1 + (nt // nh) * h2
                                sn = nt // nh
                                nc.tensor.matmul(
                                    psP[:, vl * w + sn * 8 * h2:
                                        vl * w + sn * 8 * (h2 + 1)],
                                    Wds[:, k * VSP + vc * 128:
                                        k * VSP + (vc + 1) * 128],
                                    decv[:, s0: s0 + sn, 8 * k:8 * k + 8],
                                    start=(k == 0 and h2 == 0),
                                    stop=(k == 3 and h2 == nh - 1),
                                    skip_group_check=True)
                    # two ~256-col copy pieces per task, emitted later at
                    # engine-idle points of the chain schedule
                    copy_q.append((psP, vcg0, nvcg, t0, nt, 0, nvc // 2))
                    copy_q.append((psP, vcg0, nvcg, t0, nt, nvc // 2, nvc))

            def emit_copy(n, eng):
                for _ in range(n):
                    if not copy_q:
                        return
                    psP, vcg0, nvcg, t0, nt, vl0, vl1 = copy_q.pop(0)
                    pv4 = psP.rearrange("p (v t b) -> p v t b", t=nt, b=8)
                    src = pv4[:, vl0:vl1, :, :]
                    dst = stg[:, 4 * vcg0 + vl0:4 * vcg0 + vl1, t0:t0 + nt, :]
                    if eng == "act":
                        nc.scalar.activation(dst, src, AF.Copy)
                    else:
                        nc.vector.tensor_copy(dst, src)
                    for reg in regions:
                        if reg[0] <= t0 < reg[1]:
                            reg[3] += 1
                            if reg[3] == reg[2]:
                                nc.sync.dma_start(
                                    out_d[:, reg[0] * 256:reg[1] * 256],
                                    stage[:, reg[0] * 256:reg[1] * 256])

            # ---------- main loop ----------
            inject(0, 0)
            for t in range(T):
                s = t - 1   # layer-1 step handled this superstep
                mm_rec(0, t)
                if s >= 0:
                    mm_x(s)
                    mm_rec(1, s)
                if t + 1 < T:
                    inject(0, t + 1)
                inject(1, t)
                sig(0, t)
                if s >= 0:
                    sig(1, s)
                cupd(0, t)
                ctanh(0, t)
                if s >= 0:
                    cupd(1, s)
                    ctanh(1, s)
                hmul(0, t)
                if s >= 0:
                    hmul(1, s)
                emit_copy(1, "act")
                emit_copy(1, "dve")
                # projection task feed (keeps PE busy in the chain gaps);
                # gated on TASK_AVAIL so queued matmuls never head-of-line
                # block the PE waiting for future decT blocks.
                budget = 2 if t == 63 else 1
                while (budget and emit_proj_idx[0] < len(pq)
                       and TASK_AVAIL[emit_proj_idx[0]] <= t):
                    emit_proj(1)
                    budget -= 1
            # drain layer-1 step T-1
            s = T - 1
            mm_x(s)
            mm_rec(1, s)
            emit_proj(1)            # (62,1): data completes mid-drain
            sig(1, s)
            emit_copy(1, "act")
            emit_copy(1, "dve")
            cupd(1, s)
            ctanh(1, s)
            hmul(1, s)
            emit_proj(len(pq) - emit_proj_idx[0])
            while copy_q:
                emit_copy(1, "dve")
                emit_copy(1, "act")
    nc.finalize()
    return nc


_NC_CACHE = None


def _get_nc():
    global _NC_CACHE
    if _NC_CACHE is None:
        _NC_CACHE = _build_nc()
    return _NC_CACHE


def _host_inputs(input_ids, enc_output, h0, c0, emb, Wih0, Whh0, bih0, bhh0,
                 Wih1, Whh1, bih1, bhh1, W1, b1, W2, b2, w3, b3, Wout, bout):
    f32 = np.float32
    x = np.asarray(emb, f32)[np.asarray(input_ids).astype(np.int64)]  # [B,T,E]

    # Layer-0 input projection on the host (exact), g-gate x2, bias folded in.
    ig0 = x @ np.asarray(Wih0, f32).T + (np.asarray(bih0, f32)
                                         + np.asarray(bhh0, f32))   # [B,T,2048]
    ig0 = ig0.reshape(B, T, 4, 4, 128)          # [b,t,g,j,x]
    ig0[:, :, 2] *= 2.0
    ig0T = ig0.transpose(2, 3, 0, 1, 4).reshape(128, T * 128)  # [(g,j,b),(t,x)]

    b1v = (np.asarray(bih1, f32) + np.asarray(bhh1, f32)).reshape(4, 4, 128)
    b1v = b1v.copy()
    b1v[2] *= 2.0                               # [g,j,x]
    b1T = np.broadcast_to(b1v[:, :, None, :], (4, 4, 8, 128)).reshape(128, 128)

    def h0T(hl):
        return hl.T.reshape(4, 128, 8).transpose(1, 0, 2).reshape(128, 32)

    c0a = (np.asarray(c0, f32) * 0.5).reshape(2, 8, 4, 128)
    c0T = c0a.transpose(3, 0, 2, 1).reshape(128, 64)  # [x, (layer,j,b)]

    # collapsed attention (exact in real arithmetic; see module docstring)
    u = np.asarray(W2, f32).T @ np.asarray(w3, f32)[0]
    ue = np.asarray(W1, f32)[:, :H].T @ u
    sc = np.asarray(enc_output, f32) @ ue                  # [B,S]
    sc = sc - sc.max(-1, keepdims=True)
    a = np.exp(sc)
    a /= a.sum(-1, keepdims=True)
    ctxh = np.einsum('bs,bsh->bh', a, np.asarray(enc_output, f32))  # [B,H]

    Wo_full = np.asarray(Wout, f32)                        # [V, 2H]
    bo_full = np.asarray(bout, f32)
    # t-independent half of the projection, added on the host
    ctxadd = ctxh @ Wo_full[:, H:].T + bo_full             # [B, V]

    Wrec = _reorder_w(np.asarray(Wih0, f32), np.asarray(Whh0, f32))
    misc = np.concatenate([np.eye(128, dtype=f32), b1T,
                           h0T(np.asarray(h0, f32)[0]),
                           h0T(np.asarray(h0, f32)[1]),
                           ig0T[:, 0:128], c0T], axis=1)
    base = {
        "W0": np.ascontiguousarray(Wrec[:, 4 * 2048:]).astype(_F8 if USE_FP8 else _BF16),
        "W1": _reorder_w(np.asarray(Wih1, f32),
                         np.asarray(Whh1, f32)).astype(_F8 if USE_FP8 else _BF16),
        "ig0": ig0T.astype(_BF16),
        "misc": misc.astype(_BF16),
    }
    maps = []
    for k in range(NCORES):
        lo = k * VS
        sh = np.zeros((VSP, H), f32)
        n = min(VSP, V - lo)
        sh[:n] = Wo_full[lo:lo + n, :H]
        t_ = sh.reshape(NVC, 128, 4, 128).transpose(3, 2, 0, 1)  # [p,k,vc,m]
        m = dict(base)
        m["Wd"] = np.ascontiguousarray(t_.reshape(128, 4 * VSP)).astype(_BF16)
        maps.append(m)
    return maps, ctxadd


def kernel(**inputs):
    from concourse.bass_utils import run_bass_kernel_spmd
    nc = _get_nc()
    maps, ctxadd = _host_inputs(**inputs)
    res = run_bass_kernel_spmd(nc, maps, list(range(NCORES))).results
    full = np.zeros((B, T, V), np.float32)
    for k in range(NCORES):
        o = np.asarray(res[k]["out"], np.float32)   # [128, T*256]
        o = o.reshape(128, T, NVC, 8)               # [x, t, vc, b]
        o = o.transpose(3, 1, 2, 0).reshape(B, T, VSP)
        n = min(VS, V - k * VS)
        full[:, :, k * VS:k * VS + n] = o[:, :, :n]
    full += ctxadd[:, None, :]
    return full
